# revision 9
# baseline (speedup 1.0000x reference)
"""Self-contained Trainium2 Bass kernel for the 2-layer decoder model
(nn_DecoderModel_4217657884693).

Sharding: DP2 x TP4. Cores 0-3 handle batch 0, cores 4-7 batch 1.
Within a TP group of 4: q heads 8/core, kv heads 2/core, MLP ff/4,
AllReduce over the group after o_proj and down_proj.

Layout: activations kept transposed (hidden dim on partitions, tokens on
the free axis). Scores are computed as S^T = K^T-major so softmax
reductions become matmuls (ones-augmented V gives rowsums for free).

Dtypes: residual fp32; q/k/v projections + scores + P@V in float32r
(full-speed matmul, ~1.4e-4 err); o/gate/up/down in bf16 (halves the
heavy weight DMA).
"""

import sys
sys.path.insert(0, '/opt/trn_rl_repo')

import numpy as np

# model dims (hardcoded per spec)
H = 2048; NH = 32; NKV = 8; HD = 64; FF = 8192; L = 2; V = 32000
B = 2; S = 1024; MAXPOS = 2048; EPS = 1e-5
NEG = float(np.finfo(np.float16).min)  # -65504.0

NCORES = 8
TP = 4
P = 128
BLK = 512                  # token block
NBLK = S // BLK            # 2
KT = H // P                # 16 hidden k-tiles
QH = NH // TP              # 8 q heads/core
KVH = NKV // TP            # 2 kv heads/core
QD = QH * HD               # 512 q dims/core
KVD = KVH * HD             # 128 kv dims/core
QMT = QD // P              # 4 q out tiles
FFS = FF // TP             # 2048 ff dims/core
FKT = FFS // P             # 16 ff k-tiles
SKT = S // P               # 8 sequence k-tiles

HEAVY_BF16 = True          # o/gate/up/down in bf16 (else f32r)
AR_BF16 = False            # allreduce payload dtype
DEBUG_TAPS = False         # extra debug outputs

_BUILT = {}


def _build():
    import concourse.bass as bass
    import concourse.tile as tile
    from concourse import bacc, mybir

    F32 = mybir.dt.float32
    F32R = mybir.dt.float32r
    BF16 = mybir.dt.bfloat16
    AF = mybir.ActivationFunctionType
    ALU = mybir.AluOpType
    ds, ts = bass.ds, bass.ts

    HVY = BF16 if HEAVY_BF16 else F32R
    ARD = BF16 if AR_BF16 else F32

    nc = bacc.Bacc("TRN2", target_bir_lowering=False, debug=False,
                   num_devices=NCORES)

    # ---------------- DRAM I/O ----------------
    X0T = nc.dram_tensor("X0T", [KT, P, S], F32, kind="ExternalInput")
    COS = nc.dram_tensor("COS", [P, S], F32, kind="ExternalInput")
    SIN = nc.dram_tensor("SIN", [P, S], F32, kind="ExternalInput")
    SINNEG = nc.dram_tensor("SINNEG", [P, S], F32, kind="ExternalInput")
    PADB = nc.dram_tensor("PADB", [P, SKT], F32, kind="ExternalInput")
    NORMW = nc.dram_tensor("NORMW", [P, KT], F32, kind="ExternalInput")
    WQ = nc.dram_tensor("WQ", [L, KT, P, QD], F32R, kind="ExternalInput")
    WK = nc.dram_tensor("WK", [L, KT, P, KVD], F32R, kind="ExternalInput")
    WV = nc.dram_tensor("WV", [L, KT, P, KVD], F32R, kind="ExternalInput")
    WO = nc.dram_tensor("WO", [L, QMT, P, H], HVY, kind="ExternalInput")
    WG = nc.dram_tensor("WG", [L, KT, P, FFS], HVY, kind="ExternalInput")
    WU = nc.dram_tensor("WU", [L, KT, P, FFS], HVY, kind="ExternalInput")
    WD = nc.dram_tensor("WD", [L, FKT, P, H], HVY, kind="ExternalInput")

    XOUT = nc.dram_tensor("XOUT", [KT, P, S], F32, kind="ExternalOutput")
    if DEBUG_TAPS:
        ADBG = nc.dram_tensor("ADBG", [P, QMT, S], HVY, kind="ExternalOutput")
        XDBG = nc.dram_tensor("XDBG", [KT, P, S], F32, kind="ExternalOutput")
        QDBG = nc.dram_tensor("QDBG", [P, QMT, S], F32R, kind="ExternalOutput")
        PDBG = nc.dram_tensor("PDBG", [P, SKT, S], F32R, kind="ExternalOutput")
        VADBG = nc.dram_tensor("VADBG", [P, SKT, 2 * (HD + 1)], F32R, kind="ExternalOutput")
        NDBG = nc.dram_tensor("NDBG", [P, SKT, S], F32, kind="ExternalOutput")
    KOUT = nc.dram_tensor("KOUT", [L, P, S], F32, kind="ExternalOutput")
    VOUT = nc.dram_tensor("VOUT", [L, P, S], F32, kind="ExternalOutput")

    # inline consts (fp32 bits, bitcast on DMA where f32r is needed)
    ident_h = nc.inline_tensor(np.eye(P, dtype=np.float32), "identc")
    ones_col_h = nc.inline_tensor(np.ones((P, 1), dtype=np.float32), "onescolc")
    ones_row_h = nc.inline_tensor(np.ones((1, P), dtype=np.float32), "onesrowc")
    maskdat = np.zeros((4, P, BLK), dtype=np.float32)
    for d in range(4):
        for ki in range(P):
            cut = 128 * d + ki
            if cut > 0:
                maskdat[d, ki, :min(cut, BLK)] = NEG
    mask_h = nc.inline_tensor(np.ascontiguousarray(maskdat.transpose(1, 0, 2)), "maskc")
    vones_h = nc.inline_tensor(np.ones((P, SKT, 1), dtype=np.float32), "vonesc")

    rg = [[0, 1, 2, 3], [4, 5, 6, 7]]

    with tile.TileContext(nc) as tc:
        import contextlib
        ctx = contextlib.ExitStack()
        with ctx:
            const = ctx.enter_context(tc.tile_pool(name="const", bufs=1))
            big = ctx.enter_context(tc.tile_pool(name="big", bufs=1))
            wpool = ctx.enter_context(tc.tile_pool(name="wpool", bufs=2))
            htp = ctx.enter_context(tc.tile_pool(name="htp", bufs=2))
            small = ctx.enter_context(tc.tile_pool(name="small", bufs=2))
            tiny = ctx.enter_context(tc.tile_pool(name="tiny", bufs=2))
            ppool = ctx.enter_context(tc.tile_pool(name="ppool", bufs=2))
            arp = ctx.enter_context(tc.tile_pool(name="arp", bufs=2))
            psAcc = ctx.enter_context(tc.tile_pool(name="psAcc", bufs=1, space="PSUM"))
            psAux = ctx.enter_context(tc.tile_pool(name="psAux", bufs=2, space="PSUM"))
            psS = ctx.enter_context(tc.tile_pool(name="psS", bufs=2, space="PSUM"))
            dram = ctx.enter_context(tc.tile_pool(name="dram", bufs=2, space="DRAM"))

            # ---- constants ----
            ident = const.tile([P, P], F32R, tag="ident")
            nc.sync.dma_start(ident[:], ident_h.ap().bitcast(F32R))
            ones_col = const.tile([P, 1], F32R, tag="ones_col")
            nc.sync.dma_start(ones_col[:], ones_col_h.ap().bitcast(F32R))
            ones_row = const.tile([1, P], F32R, tag="ones_row")
            nc.sync.dma_start(ones_row[:], ones_row_h.ap().bitcast(F32R))
            mask_sb = const.tile([P, 4, BLK], F32R, tag="mask")
            nc.sync.dma_start(mask_sb[:], mask_h.ap().bitcast(F32R))
            cos_sb = const.tile([P, S], F32, tag="cos")
            nc.sync.dma_start(cos_sb[:], COS[:])
            sin_sb = const.tile([P, S], F32, tag="sin")
            nc.sync.dma_start(sin_sb[:], SIN[:])
            sinneg_sb = const.tile([P, S], F32, tag="sinneg")
            nc.sync.dma_start(sinneg_sb[:], SINNEG[:])
            pad_sb = const.tile([P, SKT], F32, tag="pad")
            nc.sync.dma_start(pad_sb[:], PADB[:])
            normw_sb = const.tile([P, KT], F32, tag="normw")
            nc.sync.dma_start(normw_sb[:], NORMW[:])

            # ---- persistent state ----
            xT = big.tile([P, KT, S], F32, tag="xT")
            for kt in range(KT):
                nc.sync.dma_start(xT[:, kt, :], X0T[kt])
            h2T = big.tile([P, KT, BLK], HVY, tag="h2T")     # norm2 out (MLP in)
            mm_r = big.tile([P, FKT, BLK], HVY, tag="mm_r")  # silu(g)*u (down in)
            gs = big.tile([P, 4, BLK], F32R, tag="gs")       # silu(g) chunk
            kT_r = big.tile([P, S], F32R, tag="kT_r")        # roped k (score lhsT)
            qT_r = big.tile([P, QMT, BLK], F32R, tag="qT_r")
            attnT = big.tile([P, QMT, BLK], HVY, tag="attnT")
            vaug = big.tile([P, SKT, 2 * (HD + 1)], F32R, tag="vaug")
            nc.sync.dma_start(vaug[:, :, HD:HD + 1], vones_h.ap().bitcast(F32R))
            nc.sync.dma_start(vaug[:, :, 2 * HD + 1:2 * HD + 2],
                              vones_h.ap().bitcast(F32R))
            rs_t = small.tile([P, BLK], F32, tag="rs")

            def compute_rs(blk):
                """rs_t = rsqrt(mean over H of xT^2 + eps) for token block."""
                tb = ds(blk * BLK, BLK)
                acc = small.tile([P, BLK], F32, tag="tA")
                sqk = small.tile([P, BLK], F32, tag="tB")
                nc.vector.tensor_mul(acc[:], xT[:, 0, tb], xT[:, 0, tb])
                for kt in range(1, KT):
                    nc.vector.tensor_mul(sqk[:], xT[:, kt, tb], xT[:, kt, tb])
                    nc.vector.tensor_add(acc[:], acc[:], sqk[:])
                sq_r = small.tile([P, BLK], F32R, tag="tC")
                nc.scalar.copy(sq_r[:], acc[:])
                ms_ps = psAux.tile([1, BLK], F32, tag="aux")
                nc.tensor.matmul(ms_ps[:], lhsT=ones_col[:], rhs=sq_r[:],
                                 start=True, stop=True)
                ms_r = tiny.tile([1, BLK], F32R, tag="r1")
                nc.scalar.copy(ms_r[:], ms_ps[:])
                bc_ps = psAux.tile([P, BLK], F32, tag="aux")
                nc.tensor.matmul(bc_ps[:], lhsT=ones_row[:], rhs=ms_r[:],
                                 start=True, stop=True)
                t1 = small.tile([P, BLK], F32, tag="tA")
                nc.vector.tensor_scalar(out=t1[:], in0=bc_ps[:], scalar1=1.0 / H,
                                        scalar2=EPS, op0=ALU.mult, op1=ALU.add)
                t2 = small.tile([P, BLK], F32, tag="tB")
                nc.vector.reciprocal(t2[:], t1[:])
                nc.scalar.sqrt(rs_t[:], t2[:])

            def rope_evict(ps, blk, out_f32, out_r):
                """rope a [128, BLK] psum tile; out_f32 (fp32, optional) and
                out_r (f32r via ACT)."""
                tb = ds(blk * BLK, BLK)
                t1 = small.tile([P, BLK], F32, tag="tA")
                t2s = small.tile([P, BLK], F32, tag="tB")
                nc.vector.tensor_mul(t1[:], ps[:], cos_sb[:, tb])
                for half in range(2):
                    b0 = 64 * half
                    nc.vector.tensor_mul(t2s[b0:b0 + 32, :], ps[b0 + 32:b0 + 64, :],
                                         sinneg_sb[b0 + 32:b0 + 64, tb])
                    nc.vector.tensor_mul(t2s[b0 + 32:b0 + 64, :], ps[b0:b0 + 32, :],
                                         sin_sb[b0:b0 + 32, tb])
                tgt = out_f32
                if tgt is None:
                    tgt = small.tile([P, BLK], F32, tag="tC")
                nc.vector.tensor_add(tgt[:], t1[:], t2s[:])
                nc.scalar.copy(out_r, tgt[:])

            # ================ layers ================
            for l in range(L):
                for blk in range(NBLK):
                    tb = ds(blk * BLK, BLK)
                    # ---- norm1 + qkv (kt-outer, hT just-in-time) ----
                    compute_rs(blk)
                    ps_q = [psAcc.tile([P, BLK], F32, tag=f"acc{m}", name=f"psq{m}")
                            for m in range(QMT)]
                    ps_k = psAux.tile([P, BLK], F32, tag="aux")
                    ps_v = psAux.tile([P, BLK], F32, tag="aux")
                    for kt in range(KT):
                        hf = small.tile([P, BLK], F32, tag="tC")
                        nc.vector.tensor_mul(hf[:], xT[:, kt, tb], rs_t[:])
                        htk = htp.tile([P, BLK], F32R, tag="htk")
                        nc.scalar.copy(htk[:], hf[:])
                        wq_sb = wpool.tile([P, QD], F32R, tag="wq")
                        nc.sync.dma_start(wq_sb[:], WQ[l, kt])
                        wk_sb = wpool.tile([P, KVD], F32R, tag="wk")
                        nc.sync.dma_start(wk_sb[:], WK[l, kt])
                        wv_sb = wpool.tile([P, KVD], F32R, tag="wv")
                        nc.sync.dma_start(wv_sb[:], WV[l, kt])
                        st = (kt == 0); sp = (kt == KT - 1)
                        for m in range(QMT):
                            nc.tensor.matmul(ps_q[m][:], lhsT=wq_sb[:, ts(m, P)],
                                             rhs=htk[:], start=st, stop=sp)
                        nc.tensor.matmul(ps_k[:], lhsT=wk_sb[:], rhs=htk[:],
                                         start=st, stop=sp)
                        nc.tensor.matmul(ps_v[:], lhsT=wv_sb[:], rhs=htk[:],
                                         start=st, stop=sp)
                    # rope q -> qT_r
                    for m in range(QMT):
                        rope_evict(ps_q[m], blk, None, qT_r[:, m, :])
                    # rope k -> KOUT + kT_r
                    kf32 = small.tile([P, BLK], F32, tag="kf32")
                    rope_evict(ps_k, blk, kf32, kT_r[:, tb])
                    nc.sync.dma_start(KOUT[l][:, tb], kf32[:])
                    # v -> VOUT + transpose into vaug
                    vf32 = small.tile([P, BLK], F32, tag="tA")
                    nc.vector.tensor_copy(vf32[:], ps_v[:])
                    nc.sync.dma_start(VOUT[l][:, tb], vf32[:])
                    vtr = small.tile([P, BLK], F32R, tag="vtr")
                    nc.scalar.copy(vtr[:], ps_v[:])
                    for tt in range(BLK // P):
                        pt = psAux.tile([P, P], F32R, tag="aux")
                        nc.tensor.transpose(pt[:], vtr[:, ts(tt, P)], ident[:])
                        abs_tt = blk * (BLK // P) + tt
                        for j in range(KVH):
                            nc.scalar.copy(
                                vaug[:, abs_tt, j * (HD + 1):j * (HD + 1) + HD],
                                pt[:, ts(j, HD)])
                    # ---- attention ----
                    nkt = (blk + 1) * (BLK // P)
                    for hh in range(QH):
                        kvh = hh // (QH // KVH)
                        a_ps = psAcc.tile([HD + 1, BLK], F32, tag=f"acc{hh % 4}")
                        for ktk in range(nkt):
                            d_idx = ktk - blk * (BLK // P)
                            s_ps = psS.tile([P, BLK], F32, tag="s")
                            nc.tensor.matmul(
                                s_ps[:], lhsT=kT_r[HD * kvh:HD * (kvh + 1), ts(ktk, P)],
                                rhs=qT_r[HD * kvh:HD * (kvh + 1), hh % 4, :],
                                start=True, stop=(d_idx < 0))
                            if d_idx >= 0:
                                nc.tensor.matmul(s_ps[:], lhsT=ident[:],
                                                 rhs=mask_sb[:, d_idx, :],
                                                 start=False, stop=True)
                            p_sb = ppool.tile([P, BLK], F32R, tag="p_sb")
                            nc.scalar.activation(p_sb[:], s_ps[:], AF.Exp,
                                                 bias=pad_sb[:, ktk:ktk + 1])
                            if DEBUG_TAPS and l == 0 and hh == 0:
                                nc.sync.dma_start(PDBG[:, ktk, tb], p_sb[:])
                            nc.tensor.matmul(
                                a_ps[:],
                                lhsT=vaug[:, ktk, kvh * (HD + 1):(kvh + 1) * (HD + 1)],
                                rhs=p_sb[:], start=(ktk == 0),
                                stop=(ktk == nkt - 1))
                        rsum_r = tiny.tile([1, BLK], F32R, tag="r1")
                        nc.scalar.copy(rsum_r[:], a_ps[HD:HD + 1, :])
                        rb_ps = psS.tile([P, BLK], F32, tag="s")
                        nc.tensor.matmul(rb_ps[:], lhsT=ones_row[:], rhs=rsum_r[:],
                                         start=True, stop=True)
                        rcp = small.tile([P, BLK], F32, tag="tB")
                        nc.vector.reciprocal(rcp[:], rb_ps[:])
                        if DEBUG_TAPS and l == 0 and hh == 0:
                            nc.sync.dma_start(NDBG[:, blk, tb], rcp[:])
                        nc.vector.tensor_mul(attnT[HD * kvh:HD * (kvh + 1), hh % 4, :],
                                             a_ps[0:HD, :], rcp[0:HD, :])
                    if DEBUG_TAPS and l == 0:
                        nc.sync.dma_start(ADBG[:, :, tb], attnT[:])
                        if blk == NBLK - 1:
                            nc.sync.dma_start(VADBG[:], vaug[:])
                        nc.sync.dma_start(QDBG[:, :, tb], qT_r[:])
                    # ---- o proj (chunked) + AR ----
                    cc_o_in = dram.tile([KT, P, BLK], ARD, tag="cc_o_in")
                    cc_o_out = dram.tile([KT, P, BLK], ARD, tag="cc_o_out")
                    for mch in range(4):
                        ps_o = [psAcc.tile([P, BLK], F32, tag=f"acc{m}", name=f"pso{m}")
                                for m in range(4)]
                        for kt in range(QMT):
                            wo_sb = wpool.tile([P, BLK], HVY, tag="wo")
                            nc.sync.dma_start(wo_sb[:], WO[l, kt, :, ts(mch, BLK)])
                            for m in range(4):
                                nc.tensor.matmul(ps_o[m][:],
                                                 lhsT=wo_sb[:, ts(m, P)],
                                                 rhs=attnT[:, kt, :],
                                                 start=(kt == 0),
                                                 stop=(kt == QMT - 1))
                        for m in range(4):
                            oe = ppool.tile([P, BLK], ARD, tag="oe")
                            nc.vector.tensor_copy(oe[:], ps_o[m][:])
                            nc.sync.dma_start(cc_o_in[mch * 4 + m], oe[:])
                    nc.gpsimd.collective_compute(
                        "AllReduce", ALU.add, replica_groups=rg,
                        ins=[cc_o_in.opt()], outs=[cc_o_out.opt()])
                    for m in range(KT):
                        ob = arp.tile([P, BLK], ARD, tag="ob")
                        nc.sync.dma_start(ob[:], cc_o_out[m])
                        nc.vector.tensor_add(xT[:, m, tb], xT[:, m, tb], ob[:])
                    if DEBUG_TAPS and l == 0:
                        for m in range(KT):
                            nc.sync.dma_start(XDBG[m][:, tb], xT[:, m, tb])
                    # ---- norm2 -> h2T ----
                    compute_rs(blk)
                    for kt in range(KT):
                        nc.vector.tensor_mul(h2T[:, kt, :], xT[:, kt, tb], rs_t[:])
                    # ---- MLP gate/up ----
                    for ch in range(4):
                        ps_g = [psAcc.tile([P, BLK], F32, tag=f"acc{m}", name=f"psg{m}")
                                for m in range(4)]
                        for kt in range(KT):
                            wg_sb = wpool.tile([P, BLK], HVY, tag="wg")
                            nc.sync.dma_start(wg_sb[:], WG[l, kt, :, ts(ch, BLK)])
                            for m in range(4):
                                nc.tensor.matmul(ps_g[m][:],
                                                 lhsT=wg_sb[:, ts(m, P)],
                                                 rhs=h2T[:, kt, :],
                                                 start=(kt == 0),
                                                 stop=(kt == KT - 1))
                        for m in range(4):
                            nc.scalar.activation(gs[:, m, :], ps_g[m][:], AF.Silu)
                        ps_u = [psAcc.tile([P, BLK], F32, tag=f"acc{m}", name=f"psu{m}")
                                for m in range(4)]
                        for kt in range(KT):
                            wu_sb = wpool.tile([P, BLK], HVY, tag="wu")
                            nc.sync.dma_start(wu_sb[:], WU[l, kt, :, ts(ch, BLK)])
                            for m in range(4):
                                nc.tensor.matmul(ps_u[m][:],
                                                 lhsT=wu_sb[:, ts(m, P)],
                                                 rhs=h2T[:, kt, :],
                                                 start=(kt == 0),
                                                 stop=(kt == KT - 1))
                        for m in range(4):
                            nc.vector.tensor_mul(mm_r[:, ch * 4 + m, :],
                                                 ps_u[m][:], gs[:, m, :].bitcast(F32))
                    # ---- down proj + AR ----
                    cc_d_in = dram.tile([KT, P, BLK], ARD, tag="cc_d_in")
                    cc_d_out = dram.tile([KT, P, BLK], ARD, tag="cc_d_out")
                    for mch in range(4):
                        ps_d = [psAcc.tile([P, BLK], F32, tag=f"acc{m}", name=f"psd{m}")
                                for m in range(4)]
                        for kt in range(FKT):
                            wd_sb = wpool.tile([P, BLK], HVY, tag="wd")
                            nc.sync.dma_start(wd_sb[:], WD[l, kt, :, ts(mch, BLK)])
                            for m in range(4):
                                nc.tensor.matmul(ps_d[m][:],
                                                 lhsT=wd_sb[:, ts(m, P)],
                                                 rhs=mm_r[:, kt, :],
                                                 start=(kt == 0),
                                                 stop=(kt == FKT - 1))
                        for m in range(4):
                            de = ppool.tile([P, BLK], ARD, tag="oe")
                            nc.vector.tensor_copy(de[:], ps_d[m][:])
                            nc.sync.dma_start(cc_d_in[mch * 4 + m], de[:])
                    nc.gpsimd.collective_compute(
                        "AllReduce", ALU.add, replica_groups=rg,
                        ins=[cc_d_in.opt()], outs=[cc_d_out.opt()])
                    for m in range(KT):
                        db = arp.tile([P, BLK], ARD, tag="ob")
                        nc.sync.dma_start(db[:], cc_d_out[m])
                        nc.vector.tensor_add(xT[:, m, tb], xT[:, m, tb], db[:])

            # ================ final norm ================
            for blk in range(NBLK):
                tb = ds(blk * BLK, BLK)
                compute_rs(blk)
                for kt in range(KT):
                    xo = small.tile([P, BLK], F32, tag="tC")
                    nc.vector.scalar_tensor_tensor(xo[:], xT[:, kt, tb],
                                                   normw_sb[:, kt:kt + 1], rs_t[:],
                                                   op0=ALU.mult, op1=ALU.mult)
                    nc.sync.dma_start(XOUT[kt][:, tb], xo[:])

    nc.compile()
    return nc


def _rotary_tables():
    pos = np.arange(0, HD, 2, dtype=np.float32)
    inv_freq = 1.0 / (10000.0 ** (pos / HD))
    freqs = np.outer(np.arange(MAXPOS, dtype=np.float32), inv_freq)
    emb = np.concatenate([freqs, freqs], axis=-1)
    return np.cos(emb), np.sin(emb)


def kernel(tokens, position_ids, attention_mask, use_cache, k_cache, v_cache,
           embed, ln1, ln2, Wq, Wk, Wv, Wo, Wg, Wu, Wd, norm_w):
    import ml_dtypes
    from concourse.bass_utils import run_bass_kernel_spmd

    if 'nc' not in _BUILT:
        _BUILT['nc'] = _build()
    nc = _BUILT['nc']

    tokens = np.asarray(tokens)
    position_ids = np.asarray(position_ids)
    attention_mask = np.asarray(attention_mask)
    embed = np.asarray(embed, dtype=np.float32)
    ln1 = np.asarray(ln1, dtype=np.float32)
    ln2 = np.asarray(ln2, dtype=np.float32)
    Wq = np.asarray(Wq, dtype=np.float32); Wk = np.asarray(Wk, dtype=np.float32)
    Wv = np.asarray(Wv, dtype=np.float32); Wo = np.asarray(Wo, dtype=np.float32)
    Wg = np.asarray(Wg, dtype=np.float32); Wu = np.asarray(Wu, dtype=np.float32)
    Wd = np.asarray(Wd, dtype=np.float32)
    norm_w = np.asarray(norm_w, dtype=np.float32)

    cos_t, sin_t = _rotary_tables()

    # head-interleave permutation: tile m holds (head m, head m+4)
    perm = []
    for m in range(4):
        perm += list(range(HD * m, HD * m + HD))
        perm += list(range(HD * (m + 4), HD * (m + 4) + HD))
    perm = np.array(perm)

    def hvy(x):
        return x.astype(ml_dtypes.bfloat16) if HEAVY_BF16 else x

    in_maps = []
    for c in range(NCORES):
        b, t = c // TP, c % TP
        x0 = embed[tokens[b]]                      # [S, H]
        x0T = np.ascontiguousarray(x0.T.reshape(KT, P, S))
        cosb = cos_t[position_ids[b]]              # [S, 64]
        sinb = sin_t[position_ids[b]]
        cos128 = np.ascontiguousarray(np.tile(cosb.T, (2, 1)))  # [128, S]
        sin128 = np.ascontiguousarray(np.tile(sinb.T, (2, 1)))
        padb = ((1.0 - attention_mask[b].astype(np.float32)) * NEG)
        padb = np.ascontiguousarray(padb.reshape(SKT, P).T)     # [P, SKT]
        normw = np.ascontiguousarray(norm_w.reshape(KT, P).T)   # [P, KT]

        wq = (ln1[:, :, None] * Wq) / 8.0
        wq = wq[:, :, QD * t:QD * (t + 1)][:, :, perm]
        wq = np.ascontiguousarray(wq.reshape(L, KT, P, QD))
        wk = np.ascontiguousarray(
            (ln1[:, :, None] * Wk)[:, :, KVD * t:KVD * (t + 1)]
            .reshape(L, KT, P, KVD))
        wv = np.ascontiguousarray(
            (ln1[:, :, None] * Wv)[:, :, KVD * t:KVD * (t + 1)]
            .reshape(L, KT, P, KVD))
        wo = Wo[:, QD * t:QD * (t + 1), :][:, perm, :]
        wo = np.ascontiguousarray(wo.reshape(L, QMT, P, H))
        wg = np.ascontiguousarray(
            (ln2[:, :, None] * Wg)[:, :, FFS * t:FFS * (t + 1)]
            .reshape(L, KT, P, FFS))
        wu = np.ascontiguousarray(
            (ln2[:, :, None] * Wu)[:, :, FFS * t:FFS * (t + 1)]
            .reshape(L, KT, P, FFS))
        wd = np.ascontiguousarray(
            Wd[:, FFS * t:FFS * (t + 1), :].reshape(L, FKT, P, H))

        in_maps.append({
            "X0T": x0T, "COS": cos128, "SIN": sin128,
            "SINNEG": np.ascontiguousarray(-sin128), "PADB": padb,
            "NORMW": normw,
            "WQ": wq, "WK": wk, "WV": wv,
            "WO": hvy(wo), "WG": hvy(wg), "WU": hvy(wu), "WD": hvy(wd),
        })

    res = run_bass_kernel_spmd(nc, in_maps, core_ids=list(range(NCORES)),
                               **_BUILT.get('run_kwargs', {}))
    _BUILT['last_res'] = res

    # ---- unshard ----
    xout = np.zeros((B, S, H), dtype=np.float32)
    k_out = np.zeros((L, B, NKV, S, HD), dtype=np.float32)
    v_out = np.zeros((L, B, NKV, S, HD), dtype=np.float32)
    for c in range(NCORES):
        b, t = c // TP, c % TP
        r = res.results[c]
        if t == 0:
            xo = r["XOUT"]  # [KT, P, S]
            xout[b] = xo.transpose(2, 0, 1).reshape(S, H)
        ko = r["KOUT"]  # [L, P, S]
        vo = r["VOUT"]
        for j in range(KVH):
            gh = KVH * t + j
            k_out[:, b, gh] = ko[:, HD * j:HD * (j + 1), :].transpose(0, 2, 1)
            v_out[:, b, gh] = vo[:, HD * j:HD * (j + 1), :].transpose(0, 2, 1)
    return xout, k_out, v_out


# revision 15
# speedup vs baseline: 1.0443x; 1.0443x over previous
"""Self-contained Trainium2 Bass kernel for the 2-layer decoder model
(nn_DecoderModel_4217657884693).

Sharding: DP2 x TP4. Cores 0-3 handle batch 0, cores 4-7 batch 1.
Within a TP group of 4: q heads 8/core, kv heads 2/core, MLP ff/4,
AllReduce over the group after o_proj and down_proj.

Layout: activations kept transposed (hidden dim on partitions, tokens on
the free axis). Scores are computed as S^T = K^T-major so softmax
reductions become matmuls (ones-augmented V gives rowsums for free).

Dtypes: residual fp32; q/k/v projections + scores + P@V in float32r
(full-speed matmul, ~1.4e-4 err); o/gate/up/down in bf16 (halves the
heavy weight DMA).
"""

import sys
sys.path.insert(0, '/opt/trn_rl_repo')

import numpy as np

# model dims (hardcoded per spec)
H = 2048; NH = 32; NKV = 8; HD = 64; FF = 8192; L = 2; V = 32000
B = 2; S = 1024; MAXPOS = 2048; EPS = 1e-5
NEG = float(np.finfo(np.float16).min)  # -65504.0

NCORES = 8
TP = 4
P = 128
BLK = 512                  # token block
NBLK = S // BLK            # 2
KT = H // P                # 16 hidden k-tiles
QH = NH // TP              # 8 q heads/core
KVH = NKV // TP            # 2 kv heads/core
QD = QH * HD               # 512 q dims/core
KVD = KVH * HD             # 128 kv dims/core
QMT = QD // P              # 4 q out tiles
FFS = FF // TP             # 2048 ff dims/core
FKT = FFS // P             # 16 ff k-tiles
SKT = S // P               # 8 sequence k-tiles

HEAVY_BF16 = True          # o/gate/up/down in bf16 (else f32r)
AR_BF16 = False            # allreduce payload dtype
DEBUG_TAPS = False         # extra debug outputs

_BUILT = {}


def _build():
    import concourse.bass as bass
    import concourse.tile as tile
    from concourse import bacc, mybir

    F32 = mybir.dt.float32
    F32R = mybir.dt.float32r
    BF16 = mybir.dt.bfloat16
    AF = mybir.ActivationFunctionType
    ALU = mybir.AluOpType
    ds, ts = bass.ds, bass.ts

    HVY = BF16 if HEAVY_BF16 else F32R
    ARD = BF16 if AR_BF16 else F32

    nc = bacc.Bacc("TRN2", target_bir_lowering=False, debug=False,
                   num_devices=NCORES)

    # ---------------- DRAM I/O ----------------
    X0T = nc.dram_tensor("X0T", [KT, P, S], F32, kind="ExternalInput")
    COS = nc.dram_tensor("COS", [P, S], F32, kind="ExternalInput")
    SIN = nc.dram_tensor("SIN", [P, S], F32, kind="ExternalInput")
    SINNEG = nc.dram_tensor("SINNEG", [P, S], F32, kind="ExternalInput")
    PADB = nc.dram_tensor("PADB", [P, SKT], F32, kind="ExternalInput")
    NORMW = nc.dram_tensor("NORMW", [P, KT], F32, kind="ExternalInput")
    WQ = nc.dram_tensor("WQ", [L, KT, P, QD], F32R, kind="ExternalInput")
    WK = nc.dram_tensor("WK", [L, KT, P, KVD], F32R, kind="ExternalInput")
    WV = nc.dram_tensor("WV", [L, KT, P, KVD], F32R, kind="ExternalInput")
    WO = nc.dram_tensor("WO", [L, 4, QMT, P, BLK], HVY, kind="ExternalInput")
    WG = nc.dram_tensor("WG", [L, 4, KT, P, BLK], HVY, kind="ExternalInput")
    WU = nc.dram_tensor("WU", [L, 4, KT, P, BLK], HVY, kind="ExternalInput")
    WD = nc.dram_tensor("WD", [L, 4, FKT, P, BLK], HVY, kind="ExternalInput")

    XOUT = nc.dram_tensor("XOUT", [KT, P, S], F32, kind="ExternalOutput")
    if DEBUG_TAPS:
        ADBG = nc.dram_tensor("ADBG", [P, QMT, S], HVY, kind="ExternalOutput")
        XDBG = nc.dram_tensor("XDBG", [KT, P, S], F32, kind="ExternalOutput")
        QDBG = nc.dram_tensor("QDBG", [P, QMT, S], F32R, kind="ExternalOutput")
        PDBG = nc.dram_tensor("PDBG", [P, SKT, S], F32R, kind="ExternalOutput")
        VADBG = nc.dram_tensor("VADBG", [P, SKT, 2 * (HD + 1)], F32R, kind="ExternalOutput")
        NDBG = nc.dram_tensor("NDBG", [P, SKT, S], F32, kind="ExternalOutput")
    KOUT = nc.dram_tensor("KOUT", [L, P, S], F32, kind="ExternalOutput")
    VOUT = nc.dram_tensor("VOUT", [L, P, S], F32, kind="ExternalOutput")

    # inline consts (fp32 bits, bitcast on DMA where f32r is needed)
    ident_h = nc.inline_tensor(np.eye(P, dtype=np.float32), "identc")
    ones_col_h = nc.inline_tensor(np.ones((P, 1), dtype=np.float32), "onescolc")
    ones_row_h = nc.inline_tensor(np.ones((1, P), dtype=np.float32), "onesrowc")
    maskdat = np.zeros((4, P, BLK), dtype=np.float32)
    for d in range(4):
        for ki in range(P):
            cut = 128 * d + ki
            if cut > 0:
                maskdat[d, ki, :min(cut, BLK)] = NEG
    mask_h = nc.inline_tensor(np.ascontiguousarray(maskdat.transpose(1, 0, 2)), "maskc")
    vones_h = nc.inline_tensor(np.ones((P, SKT, 1), dtype=np.float32), "vonesc")

    rg = [[0, 1, 2, 3], [4, 5, 6, 7]]

    with tile.TileContext(nc) as tc:
        import contextlib
        ctx = contextlib.ExitStack()
        with ctx:
            const = ctx.enter_context(tc.tile_pool(name="const", bufs=1))
            big = ctx.enter_context(tc.tile_pool(name="big", bufs=1))
            wpool = ctx.enter_context(tc.tile_pool(name="wpool", bufs=2))
            htp = ctx.enter_context(tc.tile_pool(name="htp", bufs=2))
            small = ctx.enter_context(tc.tile_pool(name="small", bufs=2))
            tiny = ctx.enter_context(tc.tile_pool(name="tiny", bufs=2))
            ppool = ctx.enter_context(tc.tile_pool(name="ppool", bufs=2))
            arp = ctx.enter_context(tc.tile_pool(name="arp", bufs=2))
            psAcc = ctx.enter_context(tc.tile_pool(name="psAcc", bufs=1, space="PSUM"))
            psAux = ctx.enter_context(tc.tile_pool(name="psAux", bufs=2, space="PSUM"))
            psS = ctx.enter_context(tc.tile_pool(name="psS", bufs=2, space="PSUM"))
            dram = ctx.enter_context(tc.tile_pool(name="dram", bufs=2, space="DRAM"))

            # ---- constants ----
            ident = const.tile([P, P], F32R, tag="ident")
            nc.sync.dma_start(ident[:], ident_h.ap().bitcast(F32R))
            ones_col = const.tile([P, 1], F32R, tag="ones_col")
            nc.sync.dma_start(ones_col[:], ones_col_h.ap().bitcast(F32R))
            ones_row = const.tile([1, P], F32R, tag="ones_row")
            nc.sync.dma_start(ones_row[:], ones_row_h.ap().bitcast(F32R))
            mask_sb = const.tile([P, 4, BLK], F32R, tag="mask")
            nc.sync.dma_start(mask_sb[:], mask_h.ap().bitcast(F32R))
            cos_sb = const.tile([P, S], F32, tag="cos")
            nc.sync.dma_start(cos_sb[:], COS[:])
            sin_sb = const.tile([P, S], F32, tag="sin")
            nc.sync.dma_start(sin_sb[:], SIN[:])
            sinneg_sb = const.tile([P, S], F32, tag="sinneg")
            nc.sync.dma_start(sinneg_sb[:], SINNEG[:])
            pad_sb = const.tile([P, SKT], F32, tag="pad")
            nc.sync.dma_start(pad_sb[:], PADB[:])
            normw_sb = const.tile([P, KT], F32, tag="normw")
            nc.sync.dma_start(normw_sb[:], NORMW[:])

            # ---- persistent state ----
            xT = big.tile([P, KT, S], F32, tag="xT")
            for kt in range(KT):
                nc.sync.dma_start(xT[:, kt, :], X0T[kt])
            h2T = big.tile([P, KT, BLK], HVY, tag="h2T")     # norm2 out (MLP in)
            mm_r = big.tile([P, FKT, BLK], HVY, tag="mm_r")  # silu(g)*u (down in)
            gs = big.tile([P, 4, BLK], F32R, tag="gs")       # silu(g) chunk
            kT_r = big.tile([P, S], F32R, tag="kT_r")        # roped k (score lhsT)
            qT_r = big.tile([P, QMT, BLK], F32R, tag="qT_r")
            attnT = big.tile([P, QMT, BLK], HVY, tag="attnT")
            vaug = big.tile([P, SKT, 2 * (HD + 1)], F32R, tag="vaug")
            nc.sync.dma_start(vaug[:, :, HD:HD + 1], vones_h.ap().bitcast(F32R))
            nc.sync.dma_start(vaug[:, :, 2 * HD + 1:2 * HD + 2],
                              vones_h.ap().bitcast(F32R))
            rs_t = small.tile([P, BLK], F32, tag="rs")

            def compute_rs(blk):
                """rs_t = rsqrt(mean over H of xT^2 + eps) for token block."""
                tb = ds(blk * BLK, BLK)
                acc = small.tile([P, BLK], F32, tag="tA")
                sqk = small.tile([P, BLK], F32, tag="tB")
                nc.vector.tensor_mul(acc[:], xT[:, 0, tb], xT[:, 0, tb])
                for kt in range(1, KT):
                    nc.vector.tensor_mul(sqk[:], xT[:, kt, tb], xT[:, kt, tb])
                    nc.vector.tensor_add(acc[:], acc[:], sqk[:])
                sq_r = small.tile([P, BLK], F32R, tag="tC")
                nc.scalar.copy(sq_r[:], acc[:])
                ms_ps = psAux.tile([1, BLK], F32, tag="aux")
                nc.tensor.matmul(ms_ps[:], lhsT=ones_col[:], rhs=sq_r[:],
                                 start=True, stop=True)
                ms_r = tiny.tile([1, BLK], F32R, tag="r1")
                nc.scalar.copy(ms_r[:], ms_ps[:])
                bc_ps = psAux.tile([P, BLK], F32, tag="aux")
                nc.tensor.matmul(bc_ps[:], lhsT=ones_row[:], rhs=ms_r[:],
                                 start=True, stop=True)
                t1 = small.tile([P, BLK], F32, tag="tA")
                nc.vector.tensor_scalar(out=t1[:], in0=bc_ps[:], scalar1=1.0 / H,
                                        scalar2=EPS, op0=ALU.mult, op1=ALU.add)
                t2 = small.tile([P, BLK], F32, tag="tB")
                nc.vector.reciprocal(t2[:], t1[:])
                nc.scalar.sqrt(rs_t[:], t2[:])

            def rope_evict(ps, blk, out_f32, out_r):
                """rope a [128, BLK] psum tile; out_f32 (fp32, optional) and
                out_r (f32r via ACT)."""
                tb = ds(blk * BLK, BLK)
                t1 = small.tile([P, BLK], F32, tag="tA")
                t2s = small.tile([P, BLK], F32, tag="tB")
                nc.vector.tensor_mul(t1[:], ps[:], cos_sb[:, tb])
                for half in range(2):
                    b0 = 64 * half
                    nc.vector.tensor_mul(t2s[b0:b0 + 32, :], ps[b0 + 32:b0 + 64, :],
                                         sinneg_sb[b0 + 32:b0 + 64, tb])
                    nc.vector.tensor_mul(t2s[b0 + 32:b0 + 64, :], ps[b0:b0 + 32, :],
                                         sin_sb[b0:b0 + 32, tb])
                tgt = out_f32
                if tgt is None:
                    tgt = small.tile([P, BLK], F32, tag="tC")
                nc.vector.tensor_add(tgt[:], t1[:], t2s[:])
                nc.scalar.copy(out_r, tgt[:])

            # ================ layers ================
            pend_d = {}   # blk -> cc_d_out to fold into xT before reuse

            def apply_pending(blk):
                tb = ds(blk * BLK, BLK)
                cc = pend_d.pop(blk, None)
                if cc is None:
                    return
                for m in range(KT):
                    db = arp.tile([P, BLK], ARD, tag="ob", name=f"db{m}")
                    nc.sync.dma_start(db[:], cc[m])
                    nc.vector.tensor_add(xT[:, m, tb], xT[:, m, tb], db[:])

            def phase_qkv_attn(l, blk):
                """norm1 + qkv + rope + v-transpose + attention + o-proj;
                issues the o AllReduce and returns its output dram tile."""
                tb = ds(blk * BLK, BLK)
                apply_pending(blk)
                compute_rs(blk)
                ps_q = [psAcc.tile([P, BLK], F32, tag=f"acc{m}", name=f"psq{m}")
                        for m in range(QMT)]
                ps_k = psAux.tile([P, BLK], F32, tag="aux")
                ps_v = psAux.tile([P, BLK], F32, tag="aux")
                for kt in range(KT):
                    hf = small.tile([P, BLK], F32, tag="tC")
                    nc.vector.tensor_mul(hf[:], xT[:, kt, tb], rs_t[:])
                    htk = htp.tile([P, BLK], F32R, tag="htk")
                    nc.scalar.copy(htk[:], hf[:])
                    wq_sb = wpool.tile([P, QD], F32R, tag="wq")
                    nc.gpsimd.dma_start(wq_sb[:], WQ[l, kt])
                    wk_sb = wpool.tile([P, KVD], F32R, tag="wk")
                    nc.gpsimd.dma_start(wk_sb[:], WK[l, kt])
                    wv_sb = wpool.tile([P, KVD], F32R, tag="wv")
                    nc.gpsimd.dma_start(wv_sb[:], WV[l, kt])
                    st = (kt == 0); sp = (kt == KT - 1)
                    for m in range(QMT):
                        nc.tensor.matmul(ps_q[m][:], lhsT=wq_sb[:, ts(m, P)],
                                         rhs=htk[:], start=st, stop=sp)
                    nc.tensor.matmul(ps_k[:], lhsT=wk_sb[:], rhs=htk[:],
                                     start=st, stop=sp)
                    nc.tensor.matmul(ps_v[:], lhsT=wv_sb[:], rhs=htk[:],
                                     start=st, stop=sp)
                for m in range(QMT):
                    rope_evict(ps_q[m], blk, None, qT_r[:, m, :])
                kf32 = small.tile([P, BLK], F32, tag="kf32")
                rope_evict(ps_k, blk, kf32, kT_r[:, tb])
                nc.sync.dma_start(KOUT[l][:, tb], kf32[:])
                vf32 = small.tile([P, BLK], F32, tag="tA")
                nc.vector.tensor_copy(vf32[:], ps_v[:])
                nc.sync.dma_start(VOUT[l][:, tb], vf32[:])
                vtr = small.tile([P, BLK], F32R, tag="vtr")
                nc.scalar.copy(vtr[:], ps_v[:])
                for tt in range(BLK // P):
                    pt = psAux.tile([P, P], F32R, tag="aux")
                    nc.tensor.transpose(pt[:], vtr[:, ts(tt, P)], ident[:])
                    abs_tt = blk * (BLK // P) + tt
                    for j in range(KVH):
                        nc.scalar.copy(
                            vaug[:, abs_tt, j * (HD + 1):j * (HD + 1) + HD],
                            pt[:, ts(j, HD)])
                # attention; rowsums collected for one batched reciprocal
                nkt = (blk + 1) * (BLK // P)
                rsums = big.tile([P, BLK], F32, tag="rsums")
                rcp8 = big.tile([P, BLK], F32, tag="rcp8")
                for half in range(2):
                    hh_range = range(4 * half, 4 * half + 4)
                    for hh in hh_range:
                        kvh = hh // (QH // KVH)
                        a_ps = psAcc.tile([HD + 1, BLK], F32, tag=f"acc{hh % 4}",
                                          name=f"aps{hh}")
                        for ktk in range(nkt):
                            d_idx = ktk - blk * (BLK // P)
                            s_ps = psS.tile([P, BLK], F32, tag="s")
                            nc.tensor.matmul(
                                s_ps[:],
                                lhsT=kT_r[HD * kvh:HD * (kvh + 1), ts(ktk, P)],
                                rhs=qT_r[HD * kvh:HD * (kvh + 1), hh % 4, :],
                                start=True, stop=(d_idx < 0))
                            if d_idx >= 0:
                                nc.tensor.matmul(s_ps[:], lhsT=ident[:],
                                                 rhs=mask_sb[:, d_idx, :],
                                                 start=False, stop=True)
                            p_sb = ppool.tile([P, BLK], F32R, tag="p_sb")
                            nc.scalar.activation(p_sb[:], s_ps[:], AF.Exp,
                                                 bias=pad_sb[:, ktk:ktk + 1])
                            nc.tensor.matmul(
                                a_ps[:],
                                lhsT=vaug[:, ktk, kvh * (HD + 1):(kvh + 1) * (HD + 1)],
                                rhs=p_sb[:], start=(ktk == 0),
                                stop=(ktk == nkt - 1))
                        rrow = 32 * (hh % 4)
                        nc.scalar.copy(rsums[rrow:rrow + 1, :], a_ps[HD:HD + 1, :])
                        nc.vector.tensor_copy(
                            attnT[HD * kvh:HD * (kvh + 1), hh % 4, :], a_ps[0:HD, :])
                    nc.vector.reciprocal(rcp8[:], rsums[:])
                    for hh in hh_range:
                        kvh = hh // (QH // KVH)
                        r1 = tiny.tile([1, BLK], F32R, tag="r1")
                        nc.scalar.copy(r1[:], rcp8[32 * (hh % 4):32 * (hh % 4) + 1, :])
                        rb_ps = psS.tile([P, BLK], F32, tag="s")
                        nc.tensor.matmul(rb_ps[:], lhsT=ones_row[:], rhs=r1[:],
                                         start=True, stop=True)
                        sl = attnT[HD * kvh:HD * (kvh + 1), hh % 4, :]
                        nc.vector.tensor_mul(sl, sl, rb_ps[0:HD, :])
                if DEBUG_TAPS and l == 0:
                    nc.sync.dma_start(ADBG[:, :, tb], attnT[:])
                    nc.sync.dma_start(QDBG[:, :, tb], qT_r[:])
                # o proj (chunked) + AR issue
                cc_o_in = dram.tile([KT, P, BLK], ARD, tag="cc_o_in")
                cc_o_out = dram.tile([KT, P, BLK], ARD, tag="cc_o_out")
                for mch in range(4):
                    ps_o = [psAcc.tile([P, BLK], F32, tag=f"acc{m}", name=f"pso{m}")
                            for m in range(4)]
                    for kt in range(QMT):
                        wo_sb = wpool.tile([P, BLK], HVY, tag="wo")
                        nc.gpsimd.dma_start(wo_sb[:], WO[l, mch, kt])
                        for m in range(4):
                            nc.tensor.matmul(ps_o[m][:],
                                             lhsT=wo_sb[:, ts(m, P)],
                                             rhs=attnT[:, kt, :],
                                             start=(kt == 0),
                                             stop=(kt == QMT - 1))
                    for m in range(4):
                        oe = ppool.tile([P, BLK], ARD, tag="oe")
                        nc.vector.tensor_copy(oe[:], ps_o[m][:])
                        nc.sync.dma_start(cc_o_in[mch * 4 + m], oe[:])
                nc.gpsimd.collective_compute(
                    "AllReduce", ALU.add, replica_groups=rg,
                    ins=[cc_o_in.opt()], outs=[cc_o_out.opt()])
                return cc_o_out

            def phase_mlp(l, blk, cc_o_out):
                """fold o-AR into xT, norm2, gate/up/down; issues the down
                AllReduce and stores it as pending."""
                tb = ds(blk * BLK, BLK)
                for m in range(KT):
                    ob = arp.tile([P, BLK], ARD, tag="ob", name=f"ob{m}")
                    nc.sync.dma_start(ob[:], cc_o_out[m])
                    nc.vector.tensor_add(xT[:, m, tb], xT[:, m, tb], ob[:])
                if DEBUG_TAPS and l == 0:
                    for m in range(KT):
                        nc.sync.dma_start(XDBG[m][:, tb], xT[:, m, tb])
                compute_rs(blk)
                for kt in range(KT):
                    nc.vector.tensor_mul(h2T[:, kt, :], xT[:, kt, tb], rs_t[:])
                for ch in range(4):
                    ps_g = [psAcc.tile([P, BLK], F32, tag=f"acc{m}", name=f"psg{m}")
                            for m in range(4)]
                    for kt in range(KT):
                        wg_sb = wpool.tile([P, BLK], HVY, tag="wg")
                        nc.gpsimd.dma_start(wg_sb[:], WG[l, ch, kt])
                        for m in range(4):
                            nc.tensor.matmul(ps_g[m][:],
                                             lhsT=wg_sb[:, ts(m, P)],
                                             rhs=h2T[:, kt, :],
                                             start=(kt == 0),
                                             stop=(kt == KT - 1))
                    for m in range(4):
                        nc.scalar.activation(gs[:, m, :], ps_g[m][:], AF.Silu)
                    ps_u = [psAcc.tile([P, BLK], F32, tag=f"acc{m}", name=f"psu{m}")
                            for m in range(4)]
                    for kt in range(KT):
                        wu_sb = wpool.tile([P, BLK], HVY, tag="wu")
                        nc.gpsimd.dma_start(wu_sb[:], WU[l, ch, kt])
                        for m in range(4):
                            nc.tensor.matmul(ps_u[m][:],
                                             lhsT=wu_sb[:, ts(m, P)],
                                             rhs=h2T[:, kt, :],
                                             start=(kt == 0),
                                             stop=(kt == KT - 1))
                    for m in range(4):
                        nc.vector.tensor_mul(mm_r[:, ch * 4 + m, :],
                                             ps_u[m][:], gs[:, m, :].bitcast(F32))
                cc_d_in = dram.tile([KT, P, BLK], ARD, tag="cc_d_in")
                cc_d_out = dram.tile([KT, P, BLK], ARD, tag="cc_d_out")
                for mch in range(4):
                    ps_d = [psAcc.tile([P, BLK], F32, tag=f"acc{m}", name=f"psd{m}")
                            for m in range(4)]
                    for kt in range(FKT):
                        wd_sb = wpool.tile([P, BLK], HVY, tag="wd")
                        nc.gpsimd.dma_start(wd_sb[:], WD[l, mch, kt])
                        for m in range(4):
                            nc.tensor.matmul(ps_d[m][:],
                                             lhsT=wd_sb[:, ts(m, P)],
                                             rhs=mm_r[:, kt, :],
                                             start=(kt == 0),
                                             stop=(kt == FKT - 1))
                    for m in range(4):
                        de = ppool.tile([P, BLK], ARD, tag="oe")
                        nc.vector.tensor_copy(de[:], ps_d[m][:])
                        nc.sync.dma_start(cc_d_in[mch * 4 + m], de[:])
                nc.gpsimd.collective_compute(
                    "AllReduce", ALU.add, replica_groups=rg,
                    ins=[cc_d_in.opt()], outs=[cc_d_out.opt()])
                pend_d[blk] = cc_d_out

            for l in range(L):
                co0 = phase_qkv_attn(l, 0)
                co1 = phase_qkv_attn(l, 1)
                phase_mlp(l, 0, co0)
                phase_mlp(l, 1, co1)
            apply_pending(0)
            apply_pending(1)

            # ================ final norm ================
            for blk in range(NBLK):
                tb = ds(blk * BLK, BLK)
                compute_rs(blk)
                for kt in range(KT):
                    xo = small.tile([P, BLK], F32, tag="tC")
                    nc.vector.scalar_tensor_tensor(xo[:], xT[:, kt, tb],
                                                   normw_sb[:, kt:kt + 1], rs_t[:],
                                                   op0=ALU.mult, op1=ALU.mult)
                    nc.sync.dma_start(XOUT[kt][:, tb], xo[:])

    nc.compile()
    return nc


def _rotary_tables():
    pos = np.arange(0, HD, 2, dtype=np.float32)
    inv_freq = 1.0 / (10000.0 ** (pos / HD))
    freqs = np.outer(np.arange(MAXPOS, dtype=np.float32), inv_freq)
    emb = np.concatenate([freqs, freqs], axis=-1)
    return np.cos(emb), np.sin(emb)


def kernel(tokens, position_ids, attention_mask, use_cache, k_cache, v_cache,
           embed, ln1, ln2, Wq, Wk, Wv, Wo, Wg, Wu, Wd, norm_w):
    import ml_dtypes
    from concourse.bass_utils import run_bass_kernel_spmd

    if 'nc' not in _BUILT:
        _BUILT['nc'] = _build()
    nc = _BUILT['nc']

    tokens = np.asarray(tokens)
    position_ids = np.asarray(position_ids)
    attention_mask = np.asarray(attention_mask)
    embed = np.asarray(embed, dtype=np.float32)
    ln1 = np.asarray(ln1, dtype=np.float32)
    ln2 = np.asarray(ln2, dtype=np.float32)
    Wq = np.asarray(Wq, dtype=np.float32); Wk = np.asarray(Wk, dtype=np.float32)
    Wv = np.asarray(Wv, dtype=np.float32); Wo = np.asarray(Wo, dtype=np.float32)
    Wg = np.asarray(Wg, dtype=np.float32); Wu = np.asarray(Wu, dtype=np.float32)
    Wd = np.asarray(Wd, dtype=np.float32)
    norm_w = np.asarray(norm_w, dtype=np.float32)

    cos_t, sin_t = _rotary_tables()

    # head-interleave permutation: tile m holds (head m, head m+4)
    perm = []
    for m in range(4):
        perm += list(range(HD * m, HD * m + HD))
        perm += list(range(HD * (m + 4), HD * (m + 4) + HD))
    perm = np.array(perm)

    def hvy(x):
        return x.astype(ml_dtypes.bfloat16) if HEAVY_BF16 else x

    in_maps = []
    for c in range(NCORES):
        b, t = c // TP, c % TP
        x0 = embed[tokens[b]]                      # [S, H]
        x0T = np.ascontiguousarray(x0.T.reshape(KT, P, S))
        cosb = cos_t[position_ids[b]]              # [S, 64]
        sinb = sin_t[position_ids[b]]
        cos128 = np.ascontiguousarray(np.tile(cosb.T, (2, 1)))  # [128, S]
        sin128 = np.ascontiguousarray(np.tile(sinb.T, (2, 1)))
        padb = ((1.0 - attention_mask[b].astype(np.float32)) * NEG)
        padb = np.ascontiguousarray(padb.reshape(SKT, P).T)     # [P, SKT]
        normw = np.ascontiguousarray(norm_w.reshape(KT, P).T)   # [P, KT]

        wq = (ln1[:, :, None] * Wq) / 8.0
        wq = wq[:, :, QD * t:QD * (t + 1)][:, :, perm]
        wq = np.ascontiguousarray(wq.reshape(L, KT, P, QD))
        wk = np.ascontiguousarray(
            (ln1[:, :, None] * Wk)[:, :, KVD * t:KVD * (t + 1)]
            .reshape(L, KT, P, KVD))
        wv = np.ascontiguousarray(
            (ln1[:, :, None] * Wv)[:, :, KVD * t:KVD * (t + 1)]
            .reshape(L, KT, P, KVD))
        wo = Wo[:, QD * t:QD * (t + 1), :][:, perm, :]
        wo = wo.reshape(L, QMT, P, 4, BLK).transpose(0, 3, 1, 2, 4)
        wo = np.ascontiguousarray(wo)
        wg = (ln2[:, :, None] * Wg)[:, :, FFS * t:FFS * (t + 1)]
        wg = wg.reshape(L, KT, P, 4, BLK).transpose(0, 3, 1, 2, 4)
        wg = np.ascontiguousarray(wg)
        wu = (ln2[:, :, None] * Wu)[:, :, FFS * t:FFS * (t + 1)]
        wu = wu.reshape(L, KT, P, 4, BLK).transpose(0, 3, 1, 2, 4)
        wu = np.ascontiguousarray(wu)
        wd = Wd[:, FFS * t:FFS * (t + 1), :]
        wd = wd.reshape(L, FKT, P, 4, BLK).transpose(0, 3, 1, 2, 4)
        wd = np.ascontiguousarray(wd)

        in_maps.append({
            "X0T": x0T, "COS": cos128, "SIN": sin128,
            "SINNEG": np.ascontiguousarray(-sin128), "PADB": padb,
            "NORMW": normw,
            "WQ": wq, "WK": wk, "WV": wv,
            "WO": hvy(wo), "WG": hvy(wg), "WU": hvy(wu), "WD": hvy(wd),
        })

    res = run_bass_kernel_spmd(nc, in_maps, core_ids=list(range(NCORES)),
                               **_BUILT.get('run_kwargs', {}))
    _BUILT['last_res'] = res

    # ---- unshard ----
    xout = np.zeros((B, S, H), dtype=np.float32)
    k_out = np.zeros((L, B, NKV, S, HD), dtype=np.float32)
    v_out = np.zeros((L, B, NKV, S, HD), dtype=np.float32)
    for c in range(NCORES):
        b, t = c // TP, c % TP
        r = res.results[c]
        if t == 0:
            xo = r["XOUT"]  # [KT, P, S]
            xout[b] = xo.transpose(2, 0, 1).reshape(S, H)
        ko = r["KOUT"]  # [L, P, S]
        vo = r["VOUT"]
        for j in range(KVH):
            gh = KVH * t + j
            k_out[:, b, gh] = ko[:, HD * j:HD * (j + 1), :].transpose(0, 2, 1)
            v_out[:, b, gh] = vo[:, HD * j:HD * (j + 1), :].transpose(0, 2, 1)
    return xout, k_out, v_out


# revision 17
# speedup vs baseline: 1.2016x; 1.1506x over previous
"""Self-contained Trainium2 Bass kernel for the 2-layer decoder model
(nn_DecoderModel_4217657884693).

Sharding: DP2 x TP4. Cores 0-3 handle batch 0, cores 4-7 batch 1.
Within a TP group of 4: q heads 8/core, kv heads 2/core, MLP ff/4,
AllReduce over the group after o_proj and down_proj.

Layout: activations kept transposed (hidden dim on partitions, tokens on
the free axis). Scores are computed as S^T = K^T-major so softmax
reductions become matmuls (ones-augmented V gives rowsums for free).

Dtypes: residual fp32; q/k/v projections + scores + P@V in float32r
(full-speed matmul, ~1.4e-4 err); o/gate/up/down in bf16 (halves the
heavy weight DMA).
"""

import sys
sys.path.insert(0, '/opt/trn_rl_repo')

import numpy as np

# model dims (hardcoded per spec)
H = 2048; NH = 32; NKV = 8; HD = 64; FF = 8192; L = 2; V = 32000
B = 2; S = 1024; MAXPOS = 2048; EPS = 1e-5
NEG = float(np.finfo(np.float16).min)  # -65504.0

NCORES = 8
TP = 4
P = 128
BLK = 512                  # token block
NBLK = S // BLK            # 2
KT = H // P                # 16 hidden k-tiles
QH = NH // TP              # 8 q heads/core
KVH = NKV // TP            # 2 kv heads/core
QD = QH * HD               # 512 q dims/core
KVD = KVH * HD             # 128 kv dims/core
QMT = QD // P              # 4 q out tiles
FFS = FF // TP             # 2048 ff dims/core
FKT = FFS // P             # 16 ff k-tiles
SKT = S // P               # 8 sequence k-tiles

HEAVY_BF16 = True          # o/gate/up/down in bf16 (else f32r)
AR_BF16 = False            # allreduce payload dtype
DEBUG_TAPS = False         # extra debug outputs

_BUILT = {}


def _build():
    import concourse.bass as bass
    import concourse.tile as tile
    from concourse import bacc, mybir

    F32 = mybir.dt.float32
    F32R = mybir.dt.float32r
    BF16 = mybir.dt.bfloat16
    AF = mybir.ActivationFunctionType
    ALU = mybir.AluOpType
    ds, ts = bass.ds, bass.ts

    HVY = BF16 if HEAVY_BF16 else F32R
    ARD = BF16 if AR_BF16 else F32

    nc = bacc.Bacc("TRN2", target_bir_lowering=False, debug=False,
                   num_devices=NCORES)

    # ---------------- DRAM I/O ----------------
    X0T = nc.dram_tensor("X0T", [KT, P, S], F32, kind="ExternalInput")
    COS = nc.dram_tensor("COS", [P, S], F32, kind="ExternalInput")
    SIN = nc.dram_tensor("SIN", [P, S], F32, kind="ExternalInput")
    SINNEG = nc.dram_tensor("SINNEG", [P, S], F32, kind="ExternalInput")
    PADB = nc.dram_tensor("PADB", [P, SKT], F32, kind="ExternalInput")
    NORMW = nc.dram_tensor("NORMW", [P, KT], F32, kind="ExternalInput")
    WQ = nc.dram_tensor("WQ", [L, KT, P, QD], BF16, kind="ExternalInput")
    WK = nc.dram_tensor("WK", [L, KT, P, KVD], BF16, kind="ExternalInput")
    WV = nc.dram_tensor("WV", [L, KT, P, KVD], BF16, kind="ExternalInput")
    WO = nc.dram_tensor("WO", [L, 4, QMT, P, BLK], HVY, kind="ExternalInput")
    WG = nc.dram_tensor("WG", [L, 4, KT, P, BLK], HVY, kind="ExternalInput")
    WU = nc.dram_tensor("WU", [L, 4, KT, P, BLK], HVY, kind="ExternalInput")
    WD = nc.dram_tensor("WD", [L, 4, FKT, P, BLK], HVY, kind="ExternalInput")

    XOUT = nc.dram_tensor("XOUT", [KT, P, S], F32, kind="ExternalOutput")
    if DEBUG_TAPS:
        ADBG = nc.dram_tensor("ADBG", [P, QMT, S], HVY, kind="ExternalOutput")
        XDBG = nc.dram_tensor("XDBG", [KT, P, S], F32, kind="ExternalOutput")
        QDBG = nc.dram_tensor("QDBG", [P, QMT, S], F32R, kind="ExternalOutput")
        PDBG = nc.dram_tensor("PDBG", [P, SKT, S], F32R, kind="ExternalOutput")
        VADBG = nc.dram_tensor("VADBG", [P, SKT, 2 * (HD + 1)], F32R, kind="ExternalOutput")
        NDBG = nc.dram_tensor("NDBG", [P, SKT, S], F32, kind="ExternalOutput")
    KOUT = nc.dram_tensor("KOUT", [L, P, S], F32, kind="ExternalOutput")
    VOUT = nc.dram_tensor("VOUT", [L, P, S], F32, kind="ExternalOutput")

    # inline consts (fp32 bits, bitcast on DMA where f32r is needed)
    ident_h = nc.inline_tensor(np.eye(P, dtype=np.float32), "identc")
    ones_col_h = nc.inline_tensor(np.ones((P, 1), dtype=np.float32), "onescolc")
    ones_row_h = nc.inline_tensor(np.ones((1, P), dtype=np.float32), "onesrowc")
    maskdat = np.zeros((4, P, BLK), dtype=np.float32)
    for d in range(4):
        for ki in range(P):
            cut = 128 * d + ki
            if cut > 0:
                maskdat[d, ki, :min(cut, BLK)] = NEG
    mask_h = nc.inline_tensor(np.ascontiguousarray(maskdat.transpose(1, 0, 2)), "maskc")
    vones_h = nc.inline_tensor(np.ones((P, SKT, 1), dtype=np.float32), "vonesc")

    rg = [[0, 1, 2, 3], [4, 5, 6, 7]]

    with tile.TileContext(nc) as tc:
        import contextlib
        ctx = contextlib.ExitStack()
        with ctx:
            const = ctx.enter_context(tc.tile_pool(name="const", bufs=1))
            big = ctx.enter_context(tc.tile_pool(name="big", bufs=1))
            wpool = ctx.enter_context(tc.tile_pool(name="wpool", bufs=2))
            htp = ctx.enter_context(tc.tile_pool(name="htp", bufs=2))
            small = ctx.enter_context(tc.tile_pool(name="small", bufs=2))
            tiny = ctx.enter_context(tc.tile_pool(name="tiny", bufs=2))
            ppool = ctx.enter_context(tc.tile_pool(name="ppool", bufs=2))
            arp = ctx.enter_context(tc.tile_pool(name="arp", bufs=2))
            psAcc = ctx.enter_context(tc.tile_pool(name="psAcc", bufs=1, space="PSUM"))
            psAux = ctx.enter_context(tc.tile_pool(name="psAux", bufs=2, space="PSUM"))
            psS = ctx.enter_context(tc.tile_pool(name="psS", bufs=2, space="PSUM"))
            dram = ctx.enter_context(tc.tile_pool(name="dram", bufs=2, space="DRAM"))

            # ---- constants ----
            ident = const.tile([P, P], F32R, tag="ident")
            nc.sync.dma_start(ident[:], ident_h.ap().bitcast(F32R))
            ones_col = const.tile([P, 1], F32R, tag="ones_col")
            nc.sync.dma_start(ones_col[:], ones_col_h.ap().bitcast(F32R))
            ones_row = const.tile([1, P], F32R, tag="ones_row")
            nc.sync.dma_start(ones_row[:], ones_row_h.ap().bitcast(F32R))
            mask_f = const.tile([P, 4, BLK], F32, tag="mask_f")
            nc.sync.dma_start(mask_f[:], mask_h.ap())
            mask_sb = const.tile([P, 4, BLK], BF16, tag="mask")
            nc.vector.tensor_copy(mask_sb[:], mask_f[:])
            ident_s = const.tile([P, P], BF16, tag="ident_s")
            nc.vector.tensor_copy(ident_s[:], ident[:].bitcast(F32))
            ones_row_s = const.tile([1, P], BF16, tag="ones_row_s")
            nc.vector.tensor_copy(ones_row_s[:], ones_row[:].bitcast(F32))
            cos_sb = const.tile([P, S], F32, tag="cos")
            nc.sync.dma_start(cos_sb[:], COS[:])
            sin_sb = const.tile([P, S], F32, tag="sin")
            nc.sync.dma_start(sin_sb[:], SIN[:])
            sinneg_sb = const.tile([P, S], F32, tag="sinneg")
            nc.sync.dma_start(sinneg_sb[:], SINNEG[:])
            pad_sb = const.tile([P, SKT], F32, tag="pad")
            nc.sync.dma_start(pad_sb[:], PADB[:])
            normw_sb = const.tile([P, KT], F32, tag="normw")
            nc.sync.dma_start(normw_sb[:], NORMW[:])

            # ---- persistent state ----
            xT = big.tile([P, KT, S], F32, tag="xT")
            for kt in range(KT):
                nc.sync.dma_start(xT[:, kt, :], X0T[kt])
            h2T = big.tile([P, KT, BLK], HVY, tag="h2T")     # norm2 out (MLP in)
            mm_r = big.tile([P, FKT, BLK], HVY, tag="mm_r")  # silu(g)*u (down in)
            gs = big.tile([P, 4, BLK], F32R, tag="gs")       # silu(g) chunk
            kT_r = big.tile([P, S], BF16, tag="kT_r")        # roped k (score lhsT)
            qT_r = big.tile([P, QMT, BLK], BF16, tag="qT_r")
            attnT = big.tile([P, QMT, BLK], HVY, tag="attnT")
            vaug = big.tile([P, SKT, 2 * (HD + 1)], F32R, tag="vaug")
            nc.sync.dma_start(vaug[:, :, HD:HD + 1], vones_h.ap().bitcast(F32R))
            nc.sync.dma_start(vaug[:, :, 2 * HD + 1:2 * HD + 2],
                              vones_h.ap().bitcast(F32R))
            rs_t = small.tile([P, BLK], F32, tag="rs")

            def compute_rs(blk):
                """rs_t = rsqrt(mean over H of xT^2 + eps) for token block."""
                tb = ds(blk * BLK, BLK)
                acc = small.tile([P, BLK], F32, tag="tA")
                sqk = small.tile([P, BLK], F32, tag="tB")
                nc.vector.tensor_mul(acc[:], xT[:, 0, tb], xT[:, 0, tb])
                for kt in range(1, KT):
                    nc.vector.tensor_mul(sqk[:], xT[:, kt, tb], xT[:, kt, tb])
                    nc.vector.tensor_add(acc[:], acc[:], sqk[:])
                sq_r = small.tile([P, BLK], F32R, tag="tC")
                nc.scalar.copy(sq_r[:], acc[:])
                ms_ps = psAux.tile([1, BLK], F32, tag="aux")
                nc.tensor.matmul(ms_ps[:], lhsT=ones_col[:], rhs=sq_r[:],
                                 start=True, stop=True)
                ms_r = tiny.tile([1, BLK], F32R, tag="r1")
                nc.scalar.copy(ms_r[:], ms_ps[:])
                bc_ps = psAux.tile([P, BLK], F32, tag="aux")
                nc.tensor.matmul(bc_ps[:], lhsT=ones_row[:], rhs=ms_r[:],
                                 start=True, stop=True)
                t1 = small.tile([P, BLK], F32, tag="tA")
                nc.vector.tensor_scalar(out=t1[:], in0=bc_ps[:], scalar1=1.0 / H,
                                        scalar2=EPS, op0=ALU.mult, op1=ALU.add)
                t2 = small.tile([P, BLK], F32, tag="tB")
                nc.vector.reciprocal(t2[:], t1[:])
                nc.scalar.sqrt(rs_t[:], t2[:])

            def rope_evict(ps, blk, out_f32, out_r):
                """rope a [128, BLK] psum tile; out_f32 (fp32, optional) and
                out_r (f32r via ACT)."""
                tb = ds(blk * BLK, BLK)
                t1 = small.tile([P, BLK], F32, tag="tA")
                t2s = small.tile([P, BLK], F32, tag="tB")
                nc.vector.tensor_mul(t1[:], ps[:], cos_sb[:, tb])
                for half in range(2):
                    b0 = 64 * half
                    nc.vector.tensor_mul(t2s[b0:b0 + 32, :], ps[b0 + 32:b0 + 64, :],
                                         sinneg_sb[b0 + 32:b0 + 64, tb])
                    nc.vector.tensor_mul(t2s[b0 + 32:b0 + 64, :], ps[b0:b0 + 32, :],
                                         sin_sb[b0:b0 + 32, tb])
                if out_f32 is not None:
                    nc.vector.tensor_add(out_f32[:], t1[:], t2s[:])
                    nc.vector.tensor_copy(out_r, out_f32[:])
                else:
                    nc.vector.tensor_add(out_r, t1[:], t2s[:])

            # ================ layers ================
            pend_d = {}   # blk -> cc_d_out to fold into xT before reuse

            def apply_pending(blk):
                tb = ds(blk * BLK, BLK)
                cc = pend_d.pop(blk, None)
                if cc is None:
                    return
                for m in range(KT):
                    db = arp.tile([P, BLK], ARD, tag="ob", name=f"db{m}")
                    nc.sync.dma_start(db[:], cc[m])
                    nc.vector.tensor_add(xT[:, m, tb], xT[:, m, tb], db[:])

            def phase_qkv_attn(l, blk):
                """norm1 + qkv + rope + v-transpose + attention + o-proj;
                issues the o AllReduce and returns its output dram tile."""
                tb = ds(blk * BLK, BLK)
                apply_pending(blk)
                compute_rs(blk)
                ps_q = [psAcc.tile([P, BLK], F32, tag=f"acc{m}", name=f"psq{m}")
                        for m in range(QMT)]
                ps_k = psAux.tile([P, BLK], F32, tag="aux")
                ps_v = psAux.tile([P, BLK], F32, tag="aux")
                for kt in range(KT):
                    htk = htp.tile([P, BLK], BF16, tag="htk")
                    nc.vector.tensor_mul(htk[:], xT[:, kt, tb], rs_t[:])
                    wq_sb = wpool.tile([P, QD], BF16, tag="wq")
                    nc.sync.dma_start(wq_sb[:], WQ[l, kt])
                    wk_sb = wpool.tile([P, KVD], BF16, tag="wk")
                    nc.sync.dma_start(wk_sb[:], WK[l, kt])
                    wv_sb = wpool.tile([P, KVD], BF16, tag="wv")
                    nc.sync.dma_start(wv_sb[:], WV[l, kt])
                    st = (kt == 0); sp = (kt == KT - 1)
                    for m in range(QMT):
                        nc.tensor.matmul(ps_q[m][:], lhsT=wq_sb[:, ts(m, P)],
                                         rhs=htk[:], start=st, stop=sp)
                    nc.tensor.matmul(ps_k[:], lhsT=wk_sb[:], rhs=htk[:],
                                     start=st, stop=sp)
                    nc.tensor.matmul(ps_v[:], lhsT=wv_sb[:], rhs=htk[:],
                                     start=st, stop=sp)
                for m in range(QMT):
                    rope_evict(ps_q[m], blk, None, qT_r[:, m, :])
                kf32 = small.tile([P, BLK], F32, tag="kf32")
                rope_evict(ps_k, blk, kf32, kT_r[:, tb])
                nc.sync.dma_start(KOUT[l][:, tb], kf32[:])
                vf32 = small.tile([P, BLK], F32, tag="tA")
                nc.vector.tensor_copy(vf32[:], ps_v[:])
                nc.sync.dma_start(VOUT[l][:, tb], vf32[:])
                vtr = small.tile([P, BLK], F32R, tag="vtr")
                nc.scalar.copy(vtr[:], ps_v[:])
                for tt in range(BLK // P):
                    pt = psAux.tile([P, P], F32R, tag="aux")
                    nc.tensor.transpose(pt[:], vtr[:, ts(tt, P)], ident[:])
                    abs_tt = blk * (BLK // P) + tt
                    for j in range(KVH):
                        nc.scalar.copy(
                            vaug[:, abs_tt, j * (HD + 1):j * (HD + 1) + HD],
                            pt[:, ts(j, HD)])
                # attention; rowsums collected for one batched reciprocal
                nkt = (blk + 1) * (BLK // P)
                rsums = big.tile([P, BLK], F32, tag="rsums")
                rcp8 = big.tile([P, BLK], F32, tag="rcp8")
                for half in range(2):
                    hh_range = range(4 * half, 4 * half + 4)
                    for hh in hh_range:
                        kvh = hh // (QH // KVH)
                        a_ps = psAcc.tile([HD + 1, BLK], F32, tag=f"acc{hh % 4}",
                                          name=f"aps{hh}")
                        for ktk in range(nkt):
                            d_idx = ktk - blk * (BLK // P)
                            s_ps = psS.tile([P, BLK], F32, tag="s")
                            nc.tensor.matmul(
                                s_ps[:],
                                lhsT=kT_r[HD * kvh:HD * (kvh + 1), ts(ktk, P)],
                                rhs=qT_r[HD * kvh:HD * (kvh + 1), hh % 4, :],
                                start=True, stop=(d_idx < 0))
                            if d_idx >= 0:
                                nc.tensor.matmul(s_ps[:], lhsT=ident_s[:],
                                                 rhs=mask_sb[:, d_idx, :],
                                                 start=False, stop=True)
                            p_sb = ppool.tile([P, BLK], F32R, tag="p_sb")
                            nc.scalar.activation(p_sb[:], s_ps[:], AF.Exp,
                                                 bias=pad_sb[:, ktk:ktk + 1])
                            nc.tensor.matmul(
                                a_ps[:],
                                lhsT=vaug[:, ktk, kvh * (HD + 1):(kvh + 1) * (HD + 1)],
                                rhs=p_sb[:], start=(ktk == 0),
                                stop=(ktk == nkt - 1))
                        rrow = 32 * (hh % 4)
                        nc.scalar.copy(rsums[rrow:rrow + 1, :], a_ps[HD:HD + 1, :])
                        nc.vector.tensor_copy(
                            attnT[HD * kvh:HD * (kvh + 1), hh % 4, :], a_ps[0:HD, :])
                    nc.vector.reciprocal(rcp8[:], rsums[:])
                    for hh in hh_range:
                        kvh = hh // (QH // KVH)
                        r1 = tiny.tile([1, BLK], BF16, tag="r1")
                        nc.vector.tensor_copy(r1[:], rcp8[32 * (hh % 4):32 * (hh % 4) + 1, :])
                        rb_ps = psS.tile([P, BLK], F32, tag="s")
                        nc.tensor.matmul(rb_ps[:], lhsT=ones_row_s[:], rhs=r1[:],
                                         start=True, stop=True)
                        sl = attnT[HD * kvh:HD * (kvh + 1), hh % 4, :]
                        nc.vector.tensor_mul(sl, sl, rb_ps[0:HD, :])
                if DEBUG_TAPS and l == 0:
                    nc.sync.dma_start(ADBG[:, :, tb], attnT[:])
                    nc.sync.dma_start(QDBG[:, :, tb], qT_r[:])
                # o proj (chunked) + AR issue
                cc_o_in = dram.tile([KT, P, BLK], ARD, tag="cc_o_in")
                cc_o_out = dram.tile([KT, P, BLK], ARD, tag="cc_o_out")
                for mch in range(4):
                    ps_o = [psAcc.tile([P, BLK], F32, tag=f"acc{m}", name=f"pso{m}")
                            for m in range(4)]
                    for kt in range(QMT):
                        wo_sb = wpool.tile([P, BLK], HVY, tag="wo")
                        nc.sync.dma_start(wo_sb[:], WO[l, mch, kt])
                        for m in range(4):
                            nc.tensor.matmul(ps_o[m][:],
                                             lhsT=wo_sb[:, ts(m, P)],
                                             rhs=attnT[:, kt, :],
                                             start=(kt == 0),
                                             stop=(kt == QMT - 1))
                    for m in range(4):
                        oe = ppool.tile([P, BLK], ARD, tag="oe")
                        nc.vector.tensor_copy(oe[:], ps_o[m][:])
                        nc.sync.dma_start(cc_o_in[mch * 4 + m], oe[:])
                nc.gpsimd.collective_compute(
                    "AllReduce", ALU.add, replica_groups=rg,
                    ins=[cc_o_in.opt()], outs=[cc_o_out.opt()])
                return cc_o_out

            def phase_mlp(l, blk, cc_o_out):
                """fold o-AR into xT, norm2, gate/up/down; issues the down
                AllReduce and stores it as pending."""
                tb = ds(blk * BLK, BLK)
                for m in range(KT):
                    ob = arp.tile([P, BLK], ARD, tag="ob", name=f"ob{m}")
                    nc.sync.dma_start(ob[:], cc_o_out[m])
                    nc.vector.tensor_add(xT[:, m, tb], xT[:, m, tb], ob[:])
                if DEBUG_TAPS and l == 0:
                    for m in range(KT):
                        nc.sync.dma_start(XDBG[m][:, tb], xT[:, m, tb])
                compute_rs(blk)
                for kt in range(KT):
                    nc.vector.tensor_mul(h2T[:, kt, :], xT[:, kt, tb], rs_t[:])
                for ch in range(4):
                    ps_g = [psAcc.tile([P, BLK], F32, tag=f"acc{m}", name=f"psg{m}")
                            for m in range(4)]
                    for kt in range(KT):
                        wg_sb = wpool.tile([P, BLK], HVY, tag="wg")
                        nc.sync.dma_start(wg_sb[:], WG[l, ch, kt])
                        for m in range(4):
                            nc.tensor.matmul(ps_g[m][:],
                                             lhsT=wg_sb[:, ts(m, P)],
                                             rhs=h2T[:, kt, :],
                                             start=(kt == 0),
                                             stop=(kt == KT - 1))
                    for m in range(4):
                        nc.scalar.activation(gs[:, m, :], ps_g[m][:], AF.Silu)
                    ps_u = [psAcc.tile([P, BLK], F32, tag=f"acc{m}", name=f"psu{m}")
                            for m in range(4)]
                    for kt in range(KT):
                        wu_sb = wpool.tile([P, BLK], HVY, tag="wu")
                        nc.sync.dma_start(wu_sb[:], WU[l, ch, kt])
                        for m in range(4):
                            nc.tensor.matmul(ps_u[m][:],
                                             lhsT=wu_sb[:, ts(m, P)],
                                             rhs=h2T[:, kt, :],
                                             start=(kt == 0),
                                             stop=(kt == KT - 1))
                    for m in range(4):
                        nc.vector.tensor_mul(mm_r[:, ch * 4 + m, :],
                                             ps_u[m][:], gs[:, m, :].bitcast(F32))
                cc_d_in = dram.tile([KT, P, BLK], ARD, tag="cc_d_in")
                cc_d_out = dram.tile([KT, P, BLK], ARD, tag="cc_d_out")
                for mch in range(4):
                    ps_d = [psAcc.tile([P, BLK], F32, tag=f"acc{m}", name=f"psd{m}")
                            for m in range(4)]
                    for kt in range(FKT):
                        wd_sb = wpool.tile([P, BLK], HVY, tag="wd")
                        nc.sync.dma_start(wd_sb[:], WD[l, mch, kt])
                        for m in range(4):
                            nc.tensor.matmul(ps_d[m][:],
                                             lhsT=wd_sb[:, ts(m, P)],
                                             rhs=mm_r[:, kt, :],
                                             start=(kt == 0),
                                             stop=(kt == FKT - 1))
                    for m in range(4):
                        de = ppool.tile([P, BLK], ARD, tag="oe")
                        nc.vector.tensor_copy(de[:], ps_d[m][:])
                        nc.sync.dma_start(cc_d_in[mch * 4 + m], de[:])
                nc.gpsimd.collective_compute(
                    "AllReduce", ALU.add, replica_groups=rg,
                    ins=[cc_d_in.opt()], outs=[cc_d_out.opt()])
                pend_d[blk] = cc_d_out

            for l in range(L):
                co0 = phase_qkv_attn(l, 0)
                co1 = phase_qkv_attn(l, 1)
                phase_mlp(l, 0, co0)
                phase_mlp(l, 1, co1)
            apply_pending(0)
            apply_pending(1)

            # ================ final norm ================
            for blk in range(NBLK):
                tb = ds(blk * BLK, BLK)
                compute_rs(blk)
                for kt in range(KT):
                    xo = small.tile([P, BLK], F32, tag="tC")
                    nc.vector.scalar_tensor_tensor(xo[:], xT[:, kt, tb],
                                                   normw_sb[:, kt:kt + 1], rs_t[:],
                                                   op0=ALU.mult, op1=ALU.mult)
                    nc.sync.dma_start(XOUT[kt][:, tb], xo[:])

    nc.compile()
    return nc


def _rotary_tables():
    pos = np.arange(0, HD, 2, dtype=np.float32)
    inv_freq = 1.0 / (10000.0 ** (pos / HD))
    freqs = np.outer(np.arange(MAXPOS, dtype=np.float32), inv_freq)
    emb = np.concatenate([freqs, freqs], axis=-1)
    return np.cos(emb), np.sin(emb)


def kernel(tokens, position_ids, attention_mask, use_cache, k_cache, v_cache,
           embed, ln1, ln2, Wq, Wk, Wv, Wo, Wg, Wu, Wd, norm_w):
    import ml_dtypes
    from concourse.bass_utils import run_bass_kernel_spmd

    if 'nc' not in _BUILT:
        _BUILT['nc'] = _build()
    nc = _BUILT['nc']

    tokens = np.asarray(tokens)
    position_ids = np.asarray(position_ids)
    attention_mask = np.asarray(attention_mask)
    embed = np.asarray(embed, dtype=np.float32)
    ln1 = np.asarray(ln1, dtype=np.float32)
    ln2 = np.asarray(ln2, dtype=np.float32)
    Wq = np.asarray(Wq, dtype=np.float32); Wk = np.asarray(Wk, dtype=np.float32)
    Wv = np.asarray(Wv, dtype=np.float32); Wo = np.asarray(Wo, dtype=np.float32)
    Wg = np.asarray(Wg, dtype=np.float32); Wu = np.asarray(Wu, dtype=np.float32)
    Wd = np.asarray(Wd, dtype=np.float32)
    norm_w = np.asarray(norm_w, dtype=np.float32)

    cos_t, sin_t = _rotary_tables()

    # head-interleave permutation: tile m holds (head m, head m+4)
    perm = []
    for m in range(4):
        perm += list(range(HD * m, HD * m + HD))
        perm += list(range(HD * (m + 4), HD * (m + 4) + HD))
    perm = np.array(perm)

    def hvy(x):
        return x.astype(ml_dtypes.bfloat16) if HEAVY_BF16 else x

    def bf(x):
        return x.astype(ml_dtypes.bfloat16)

    in_maps = []
    for c in range(NCORES):
        b, t = c // TP, c % TP
        x0 = embed[tokens[b]]                      # [S, H]
        x0T = np.ascontiguousarray(x0.T.reshape(KT, P, S))
        cosb = cos_t[position_ids[b]]              # [S, 64]
        sinb = sin_t[position_ids[b]]
        cos128 = np.ascontiguousarray(np.tile(cosb.T, (2, 1)))  # [128, S]
        sin128 = np.ascontiguousarray(np.tile(sinb.T, (2, 1)))
        padb = ((1.0 - attention_mask[b].astype(np.float32)) * NEG)
        padb = np.ascontiguousarray(padb.reshape(SKT, P).T)     # [P, SKT]
        normw = np.ascontiguousarray(norm_w.reshape(KT, P).T)   # [P, KT]

        wq = (ln1[:, :, None] * Wq) / 8.0
        wq = wq[:, :, QD * t:QD * (t + 1)][:, :, perm]
        wq = np.ascontiguousarray(wq.reshape(L, KT, P, QD))
        wk = np.ascontiguousarray(
            (ln1[:, :, None] * Wk)[:, :, KVD * t:KVD * (t + 1)]
            .reshape(L, KT, P, KVD))
        wv = np.ascontiguousarray(
            (ln1[:, :, None] * Wv)[:, :, KVD * t:KVD * (t + 1)]
            .reshape(L, KT, P, KVD))
        wo = Wo[:, QD * t:QD * (t + 1), :][:, perm, :]
        wo = wo.reshape(L, QMT, P, 4, BLK).transpose(0, 3, 1, 2, 4)
        wo = np.ascontiguousarray(wo)
        wg = (ln2[:, :, None] * Wg)[:, :, FFS * t:FFS * (t + 1)]
        wg = wg.reshape(L, KT, P, 4, BLK).transpose(0, 3, 1, 2, 4)
        wg = np.ascontiguousarray(wg)
        wu = (ln2[:, :, None] * Wu)[:, :, FFS * t:FFS * (t + 1)]
        wu = wu.reshape(L, KT, P, 4, BLK).transpose(0, 3, 1, 2, 4)
        wu = np.ascontiguousarray(wu)
        wd = Wd[:, FFS * t:FFS * (t + 1), :]
        wd = wd.reshape(L, FKT, P, 4, BLK).transpose(0, 3, 1, 2, 4)
        wd = np.ascontiguousarray(wd)

        in_maps.append({
            "X0T": x0T, "COS": cos128, "SIN": sin128,
            "SINNEG": np.ascontiguousarray(-sin128), "PADB": padb,
            "NORMW": normw,
            "WQ": bf(wq), "WK": bf(wk), "WV": bf(wv),
            "WO": hvy(wo), "WG": hvy(wg), "WU": hvy(wu), "WD": hvy(wd),
        })

    res = run_bass_kernel_spmd(nc, in_maps, core_ids=list(range(NCORES)),
                               **_BUILT.get('run_kwargs', {}))
    _BUILT['last_res'] = res

    # ---- unshard ----
    xout = np.zeros((B, S, H), dtype=np.float32)
    k_out = np.zeros((L, B, NKV, S, HD), dtype=np.float32)
    v_out = np.zeros((L, B, NKV, S, HD), dtype=np.float32)
    for c in range(NCORES):
        b, t = c // TP, c % TP
        r = res.results[c]
        if t == 0:
            xo = r["XOUT"]  # [KT, P, S]
            xout[b] = xo.transpose(2, 0, 1).reshape(S, H)
        ko = r["KOUT"]  # [L, P, S]
        vo = r["VOUT"]
        for j in range(KVH):
            gh = KVH * t + j
            k_out[:, b, gh] = ko[:, HD * j:HD * (j + 1), :].transpose(0, 2, 1)
            v_out[:, b, gh] = vo[:, HD * j:HD * (j + 1), :].transpose(0, 2, 1)
    return xout, k_out, v_out


# revision 21
# speedup vs baseline: 1.4776x; 1.2297x over previous
"""Self-contained Trainium2 Bass kernel for the 2-layer decoder model
(nn_DecoderModel_4217657884693).

Sharding: DP2 x TP4. Cores 0-3 handle batch 0, cores 4-7 batch 1.
Within a TP group of 4: q heads 8/core, kv heads 2/core, MLP ff/4,
AllReduce over the group after o_proj and down_proj.

Layout: activations kept transposed (hidden dim on partitions, tokens on
the free axis). Scores are computed as S^T = K^T-major so softmax
reductions become matmuls (ones-augmented V gives rowsums for free).

Dtypes: residual fp32; q/k/v projections + scores + P@V in float32r
(full-speed matmul, ~1.4e-4 err); o/gate/up/down in bf16 (halves the
heavy weight DMA).
"""

import sys
sys.path.insert(0, '/opt/trn_rl_repo')

import numpy as np

# model dims (hardcoded per spec)
H = 2048; NH = 32; NKV = 8; HD = 64; FF = 8192; L = 2; V = 32000
B = 2; S = 1024; MAXPOS = 2048; EPS = 1e-5
NEG = float(np.finfo(np.float16).min)  # -65504.0

NCORES = 8
TP = 4
P = 128
BLK = 512                  # token block
NBLK = S // BLK            # 2
KT = H // P                # 16 hidden k-tiles
QH = NH // TP              # 8 q heads/core
KVH = NKV // TP            # 2 kv heads/core
QD = QH * HD               # 512 q dims/core
KVD = KVH * HD             # 128 kv dims/core
QMT = QD // P              # 4 q out tiles
FFS = FF // TP             # 2048 ff dims/core
FKT = FFS // P             # 16 ff k-tiles
SKT = S // P               # 8 sequence k-tiles

HEAVY_BF16 = True          # o/gate/up/down in bf16 (else f32r)
AR_BF16 = False            # allreduce payload dtype
DEBUG_TAPS = False         # extra debug outputs

_BUILT = {}


def _build():
    import concourse.bass as bass
    import concourse.tile as tile
    from concourse import bacc, mybir

    F32 = mybir.dt.float32
    F32R = mybir.dt.float32r
    BF16 = mybir.dt.bfloat16
    AF = mybir.ActivationFunctionType
    ALU = mybir.AluOpType
    ds, ts = bass.ds, bass.ts

    HVY = BF16 if HEAVY_BF16 else F32R
    ARD = BF16 if AR_BF16 else F32

    nc = bacc.Bacc("TRN2", target_bir_lowering=False, debug=False,
                   num_devices=NCORES)

    # ---------------- DRAM I/O ----------------
    X0T = nc.dram_tensor("X0T", [KT, P, S], F32, kind="ExternalInput")
    COS = nc.dram_tensor("COS", [P, S], F32, kind="ExternalInput")
    SIN = nc.dram_tensor("SIN", [P, S], F32, kind="ExternalInput")
    SINNEG = nc.dram_tensor("SINNEG", [P, S], F32, kind="ExternalInput")
    PADB = nc.dram_tensor("PADB", [P, SKT], F32, kind="ExternalInput")
    NORMW = nc.dram_tensor("NORMW", [P, KT], F32, kind="ExternalInput")
    WQ = nc.dram_tensor("WQ", [L, KT, P, QD], BF16, kind="ExternalInput")
    WK = nc.dram_tensor("WK", [L, KT, P, KVD], BF16, kind="ExternalInput")
    WV = nc.dram_tensor("WV", [L, KT, P, KVD], BF16, kind="ExternalInput")
    WO = nc.dram_tensor("WO", [L, 4, QMT, P, BLK], HVY, kind="ExternalInput")
    WG = nc.dram_tensor("WG", [L, 4, KT, P, BLK], HVY, kind="ExternalInput")
    WU = nc.dram_tensor("WU", [L, 4, KT, P, BLK], HVY, kind="ExternalInput")
    WD = nc.dram_tensor("WD", [L, 4, FKT, P, BLK], HVY, kind="ExternalInput")

    XOUT = nc.dram_tensor("XOUT", [KT, P, S], F32, kind="ExternalOutput")
    KOUT = nc.dram_tensor("KOUT", [L, P, S], F32, kind="ExternalOutput")
    VOUT = nc.dram_tensor("VOUT", [L, P, S], F32, kind="ExternalOutput")
    if DEBUG_TAPS:
        ADBG = nc.dram_tensor("ADBG", [P, QMT, S], HVY, kind="ExternalOutput")
        XDBG = nc.dram_tensor("XDBG", [KT, P, S], F32, kind="ExternalOutput")
        QDBG = nc.dram_tensor("QDBG", [P, QMT, S], BF16, kind="ExternalOutput")

    # inline consts (fp32 bits, bitcast on DMA where f32r is needed)
    ident_h = nc.inline_tensor(np.eye(P, dtype=np.float32), "identc")
    ones_col_h = nc.inline_tensor(np.ones((P, 1), dtype=np.float32), "onescolc")
    ones_row_h = nc.inline_tensor(np.ones((1, P), dtype=np.float32), "onesrowc")
    maskdat = np.zeros((4, P, BLK), dtype=np.float32)
    for d in range(4):
        for ki in range(P):
            cut = 128 * d + ki
            if cut > 0:
                maskdat[d, ki, :min(cut, BLK)] = NEG
    mask_h = nc.inline_tensor(np.ascontiguousarray(maskdat.transpose(1, 0, 2)), "maskc")
    vones_h = nc.inline_tensor(np.ones((P, SKT, 1), dtype=np.float32), "vonesc")

    rg = [[0, 1, 2, 3], [4, 5, 6, 7]]

    with tile.TileContext(nc) as tc:
        import contextlib
        ctx = contextlib.ExitStack()
        with ctx:
            const = ctx.enter_context(tc.tile_pool(name="const", bufs=1))
            big = ctx.enter_context(tc.tile_pool(name="big", bufs=1))
            wpool = ctx.enter_context(tc.tile_pool(name="wpool", bufs=3))
            htp = ctx.enter_context(tc.tile_pool(name="htp", bufs=3))
            small = ctx.enter_context(tc.tile_pool(name="small", bufs=2))
            tiny = ctx.enter_context(tc.tile_pool(name="tiny", bufs=2))
            ppool = ctx.enter_context(tc.tile_pool(name="ppool", bufs=2))
            arp = ctx.enter_context(tc.tile_pool(name="arp", bufs=2))
            psP = ctx.enter_context(tc.tile_pool(name="psP", bufs=1, space="PSUM"))
            dram = ctx.enter_context(tc.tile_pool(name="dram", bufs=2, space="DRAM"))

            def pst(i, shape=None, name=None, dtype=None):
                return psP.tile(shape or [P, BLK], dtype or F32, tag=f"p{i}",
                                name=name or f"pt{i}")

            # ---- constants ----
            ident = const.tile([P, P], F32R, tag="ident")
            nc.sync.dma_start(ident[:], ident_h.ap().bitcast(F32R))
            ones_col = const.tile([P, 1], F32R, tag="ones_col")
            nc.sync.dma_start(ones_col[:], ones_col_h.ap().bitcast(F32R))
            ones_row = const.tile([1, P], F32R, tag="ones_row")
            nc.sync.dma_start(ones_row[:], ones_row_h.ap().bitcast(F32R))
            ident_s = const.tile([P, P], BF16, tag="ident_s")
            nc.vector.tensor_copy(ident_s[:], ident[:].bitcast(F32))
            ones_row_s = const.tile([1, P], BF16, tag="ones_row_s")
            nc.vector.tensor_copy(ones_row_s[:], ones_row[:].bitcast(F32))
            mask_sb = const.tile([P, 4, BLK], BF16, tag="mask")
            cos_sb = const.tile([P, S], F32, tag="cos")
            nc.sync.dma_start(cos_sb[:], COS[:])
            sin_sb = const.tile([P, S], F32, tag="sin")
            nc.sync.dma_start(sin_sb[:], SIN[:])
            sinneg_sb = const.tile([P, S], F32, tag="sinneg")
            nc.sync.dma_start(sinneg_sb[:], SINNEG[:])
            pad_sb = const.tile([P, SKT], F32, tag="pad")
            nc.sync.dma_start(pad_sb[:], PADB[:])
            normw_sb = const.tile([P, KT], F32, tag="normw")
            nc.sync.dma_start(normw_sb[:], NORMW[:])

            # ---- persistent state ----
            xT = big.tile([P, KT, S], F32, tag="xT")
            for kt in range(KT):
                nc.sync.dma_start(xT[:, kt, :], X0T[kt])
            h2T = big.tile([P, KT, BLK], HVY, tag="h2T")
            mm_r = big.tile([P, FKT, BLK], HVY, tag="mm_r")
            gs = big.tile([P, 4, BLK], F32R, tag="gs")
            kT_r = big.tile([P, S], BF16, tag="kT_r")
            qT_r = big.tile([P, QMT, BLK], BF16, tag="qT_r")
            attnT = big.tile([P, QMT, BLK], HVY, tag="attnT")
            vaug = big.tile([P, SKT, 2 * (HD + 1)], F32R, tag="vaug")
            nc.sync.dma_start(vaug[:, :, HD:HD + 1], vones_h.ap().bitcast(F32R))
            nc.sync.dma_start(vaug[:, :, 2 * HD + 1:2 * HD + 2],
                              vones_h.ap().bitcast(F32R))
            rs_t = small.tile([P, BLK], F32, tag="rs")
            rsums = big.tile([P, BLK], F32, tag="rsums")
            rcp8 = big.tile([P, BLK], F32, tag="rcp8")
            for dmask in range(4):
                nc.sync.dma_start(rsums[:], mask_h.ap()[:, dmask, :])
                nc.vector.tensor_copy(mask_sb[:, dmask, :], rsums[:])

            def compute_rs(blk, b0, b1):
                """rs_t = rsqrt(mean over H of xT^2 + eps); uses psum banks
                b0 (partsum) and b1 (broadcast)."""
                tb = ds(blk * BLK, BLK)
                acc = small.tile([P, BLK], F32, tag="tA")
                sqk = small.tile([P, BLK], F32, tag="tB")
                nc.vector.tensor_mul(acc[:], xT[:, 0, tb], xT[:, 0, tb])
                for kt in range(1, KT):
                    nc.vector.tensor_mul(sqk[:], xT[:, kt, tb], xT[:, kt, tb])
                    nc.vector.tensor_add(acc[:], acc[:], sqk[:])
                sq_r = small.tile([P, BLK], F32R, tag="tC")
                nc.scalar.copy(sq_r[:], acc[:])
                ms_ps = pst(b0, [1, BLK], name="ms_ps")
                nc.tensor.matmul(ms_ps[:], lhsT=ones_col[:], rhs=sq_r[:],
                                 start=True, stop=True)
                ms_r = tiny.tile([1, BLK], F32R, tag="r1f")
                nc.scalar.copy(ms_r[:], ms_ps[:])
                bc_ps = pst(b1, name="bc_ps")
                nc.tensor.matmul(bc_ps[:], lhsT=ones_row[:], rhs=ms_r[:],
                                 start=True, stop=True)
                t1 = small.tile([P, BLK], F32, tag="tA")
                nc.vector.tensor_scalar(out=t1[:], in0=bc_ps[:], scalar1=1.0 / H,
                                        scalar2=EPS, op0=ALU.mult, op1=ALU.add)
                t2 = small.tile([P, BLK], F32, tag="tB")
                nc.vector.reciprocal(t2[:], t1[:])
                nc.scalar.sqrt(rs_t[:], t2[:])

            def rope_evict(ps, blk, out_f32, out_r):
                tb = ds(blk * BLK, BLK)
                t1 = small.tile([P, BLK], F32, tag="tA")
                t2s = small.tile([P, BLK], F32, tag="tB")
                nc.vector.tensor_mul(t1[:], ps[:], cos_sb[:, tb])
                for half in range(2):
                    b0 = 64 * half
                    nc.vector.tensor_mul(t2s[b0:b0 + 32, :], ps[b0 + 32:b0 + 64, :],
                                         sinneg_sb[b0 + 32:b0 + 64, tb])
                    nc.vector.tensor_mul(t2s[b0 + 32:b0 + 64, :], ps[b0:b0 + 32, :],
                                         sin_sb[b0:b0 + 32, tb])
                if out_f32 is not None:
                    nc.vector.tensor_add(out_f32[:], t1[:], t2s[:])
                    nc.vector.tensor_copy(out_r, out_f32[:])
                else:
                    nc.vector.tensor_add(out_r, t1[:], t2s[:])

            # ================ layers ================
            pend_d = {}

            def apply_pending(blk):
                tb = ds(blk * BLK, BLK)
                cc = pend_d.pop(blk, None)
                if cc is None:
                    return
                for m in range(KT):
                    db = arp.tile([P, BLK], ARD, tag="ob", name=f"db{m}")
                    nc.sync.dma_start(db[:], cc[m])
                    nc.vector.tensor_add(xT[:, m, tb], xT[:, m, tb], db[:])

            def phase_qkv_attn(l, blk):
                tb = ds(blk * BLK, BLK)
                apply_pending(blk)
                compute_rs(blk, 6, 7)
                # qkv: q -> banks 0-3, k -> 4, v -> 5
                ps_q = [pst(m, name=f"psq{m}") for m in range(QMT)]
                ps_k = pst(4, name="psk")
                ps_v = pst(5, name="psv")
                for kt in range(KT):
                    htk = htp.tile([P, BLK], BF16, tag="htk")
                    nc.vector.tensor_mul(htk[:], xT[:, kt, tb], rs_t[:])
                    wq_sb = wpool.tile([P, QD], BF16, tag="wq")
                    nc.sync.dma_start(wq_sb[:], WQ[l, kt])
                    wk_sb = wpool.tile([P, KVD], BF16, tag="wk")
                    nc.sync.dma_start(wk_sb[:], WK[l, kt])
                    wv_sb = wpool.tile([P, KVD], BF16, tag="wv")
                    nc.sync.dma_start(wv_sb[:], WV[l, kt])
                    st = (kt == 0); sp = (kt == KT - 1)
                    for m in range(QMT):
                        nc.tensor.matmul(ps_q[m][:], lhsT=wq_sb[:, ts(m, P)],
                                         rhs=htk[:], start=st, stop=sp)
                    nc.tensor.matmul(ps_k[:], lhsT=wk_sb[:], rhs=htk[:],
                                     start=st, stop=sp)
                    nc.tensor.matmul(ps_v[:], lhsT=wv_sb[:], rhs=htk[:],
                                     start=st, stop=sp)
                for m in range(QMT):
                    rope_evict(ps_q[m], blk, None, qT_r[:, m, :])
                kf32 = small.tile([P, BLK], F32, tag="kf32")
                rope_evict(ps_k, blk, kf32, kT_r[:, tb])
                nc.sync.dma_start(KOUT[l][:, tb], kf32[:])
                vf32 = small.tile([P, BLK], F32, tag="tA")
                nc.vector.tensor_copy(vf32[:], ps_v[:])
                nc.sync.dma_start(VOUT[l][:, tb], vf32[:])
                vtr = small.tile([P, BLK], F32R, tag="vtr")
                nc.scalar.copy(vtr[:], ps_v[:])
                for tt in range(BLK // P):
                    pt = pst(6 + tt % 2, [P, P], name=f"vtp{tt}", dtype=F32R)
                    nc.tensor.transpose(pt[:], vtr[:, ts(tt, P)], ident[:])
                    abs_tt = blk * (BLK // P) + tt
                    for j in range(KVH):
                        nc.scalar.copy(
                            vaug[:, abs_tt, j * (HD + 1):j * (HD + 1) + HD],
                            pt[:, ts(j, HD)].bitcast(F32))
                # ---- attention: 2-head bursts sharing stationaries ----
                nkt = (blk + 1) * (BLK // P)
                for half in range(2):
                    hh0 = 4 * half
                    a_ps = {hh0 + i: pst(i, [HD + 1, BLK], name=f"aps{hh0 + i}")
                            for i in range(4)}
                    for pair in range(2):
                        heads = [hh0 + 2 * pair, hh0 + 2 * pair + 1]
                        kvh = heads[0] // (QH // KVH)
                        for ktk in range(nkt):
                            d_idx = ktk - blk * (BLK // P)
                            s_ps = {}
                            for i, hh in enumerate(heads):
                                sp_t = pst(4 + (2 * pair + i) % 2, name=f"sps{hh}")
                                nc.tensor.matmul(
                                    sp_t[:],
                                    lhsT=kT_r[HD * kvh:HD * (kvh + 1), ts(ktk, P)],
                                    rhs=qT_r[HD * kvh:HD * (kvh + 1), hh % 4, :],
                                    start=True, stop=(d_idx < 0))
                                s_ps[hh] = sp_t
                            if d_idx >= 0:
                                for hh in heads:
                                    nc.tensor.matmul(s_ps[hh][:], lhsT=ident_s[:],
                                                     rhs=mask_sb[:, d_idx, :],
                                                     start=False, stop=True)
                            for hh in heads:
                                p_sb = ppool.tile([P, BLK], F32R, tag="p_sb")
                                nc.scalar.activation(p_sb[:], s_ps[hh][:], AF.Exp,
                                                     bias=pad_sb[:, ktk:ktk + 1])
                                nc.tensor.matmul(
                                    a_ps[hh][:],
                                    lhsT=vaug[:, ktk,
                                              kvh * (HD + 1):(kvh + 1) * (HD + 1)],
                                    rhs=p_sb[:], start=(ktk == 0),
                                    stop=(ktk == nkt - 1))
                        for hh in heads:
                            rrow = 32 * (hh % 4)
                            nc.scalar.copy(rsums[rrow:rrow + 1, :],
                                           a_ps[hh][HD:HD + 1, :])
                            nc.vector.tensor_copy(
                                attnT[HD * kvh:HD * (kvh + 1), hh % 4, :],
                                a_ps[hh][0:HD, :])
                    nc.vector.reciprocal(rcp8[:], rsums[:])
                    for i in range(4):
                        hh = hh0 + i
                        kvh = hh // (QH // KVH)
                        r1 = tiny.tile([1, BLK], BF16, tag="r1")
                        nc.vector.tensor_copy(r1[:], rcp8[32 * i:32 * i + 1, :])
                        rb_ps = pst(4 + i % 2, name=f"rb{hh}")
                        nc.tensor.matmul(rb_ps[:], lhsT=ones_row_s[:], rhs=r1[:],
                                         start=True, stop=True)
                        sl = attnT[HD * kvh:HD * (kvh + 1), hh % 4, :]
                        nc.vector.tensor_mul(sl, sl, rb_ps[0:HD, :])
                if DEBUG_TAPS and l == 0:
                    nc.sync.dma_start(ADBG[:, :, tb], attnT[:])
                    nc.sync.dma_start(QDBG[:, :, tb], qT_r[:])
                # ---- o proj (banks 4-7) + AR issue ----
                cc_o_in = dram.tile([KT, P, BLK], ARD, tag="cc_o_in")
                cc_o_out = dram.tile([KT, P, BLK], ARD, tag="cc_o_out")
                for mch in range(4):
                    ps_o = [pst(4 + m, name=f"pso{m}") for m in range(4)]
                    for kt in range(QMT):
                        wo_sb = wpool.tile([P, BLK], HVY, tag="wo")
                        nc.sync.dma_start(wo_sb[:], WO[l, mch, kt])
                        for m in range(4):
                            nc.tensor.matmul(ps_o[m][:],
                                             lhsT=wo_sb[:, ts(m, P)],
                                             rhs=attnT[:, kt, :],
                                             start=(kt == 0),
                                             stop=(kt == QMT - 1))
                    for m in range(4):
                        oe = ppool.tile([P, BLK], ARD, tag="oe")
                        nc.vector.tensor_copy(oe[:], ps_o[m][:])
                        nc.sync.dma_start(cc_o_in[mch * 4 + m], oe[:])
                nc.gpsimd.collective_compute(
                    "AllReduce", ALU.add, replica_groups=rg,
                    ins=[cc_o_in.opt()], outs=[cc_o_out.opt()])
                return cc_o_out

            def phase_mlp(l, blk, cc_o_out):
                tb = ds(blk * BLK, BLK)
                for m in range(KT):
                    ob = arp.tile([P, BLK], ARD, tag="ob", name=f"ob{m}")
                    nc.sync.dma_start(ob[:], cc_o_out[m])
                    nc.vector.tensor_add(xT[:, m, tb], xT[:, m, tb], ob[:])
                if DEBUG_TAPS and l == 0:
                    for m in range(KT):
                        nc.sync.dma_start(XDBG[m][:, tb], xT[:, m, tb])
                compute_rs(blk, 6, 7)
                for kt in range(KT):
                    nc.vector.tensor_mul(h2T[:, kt, :], xT[:, kt, tb], rs_t[:])
                for ch in range(4):
                    # gate -> banks 0-3, up -> banks 4-7 (no eviction wait)
                    ps_g = [pst(m, name=f"psg{m}") for m in range(4)]
                    for kt in range(KT):
                        wg_sb = wpool.tile([P, BLK], HVY, tag="wg")
                        nc.sync.dma_start(wg_sb[:], WG[l, ch, kt])
                        for m in range(4):
                            nc.tensor.matmul(ps_g[m][:],
                                             lhsT=wg_sb[:, ts(m, P)],
                                             rhs=h2T[:, kt, :],
                                             start=(kt == 0),
                                             stop=(kt == KT - 1))
                    for m in range(4):
                        nc.scalar.activation(gs[:, m, :], ps_g[m][:], AF.Silu)
                    ps_u = [pst(4 + m, name=f"psu{m}") for m in range(4)]
                    for kt in range(KT):
                        wu_sb = wpool.tile([P, BLK], HVY, tag="wu")
                        nc.sync.dma_start(wu_sb[:], WU[l, ch, kt])
                        for m in range(4):
                            nc.tensor.matmul(ps_u[m][:],
                                             lhsT=wu_sb[:, ts(m, P)],
                                             rhs=h2T[:, kt, :],
                                             start=(kt == 0),
                                             stop=(kt == KT - 1))
                    for m in range(4):
                        nc.vector.tensor_mul(mm_r[:, ch * 4 + m, :],
                                             ps_u[m][:], gs[:, m, :].bitcast(F32))
                cc_d_in = dram.tile([KT, P, BLK], ARD, tag="cc_d_in")
                cc_d_out = dram.tile([KT, P, BLK], ARD, tag="cc_d_out")
                for mch in range(4):
                    # alternate down accumulators between bank quads per chunk
                    base = 0 if mch % 2 == 0 else 4
                    ps_d = [pst(base + m, name=f"psd{m}") for m in range(4)]
                    for kt in range(FKT):
                        wd_sb = wpool.tile([P, BLK], HVY, tag="wd")
                        nc.sync.dma_start(wd_sb[:], WD[l, mch, kt])
                        for m in range(4):
                            nc.tensor.matmul(ps_d[m][:],
                                             lhsT=wd_sb[:, ts(m, P)],
                                             rhs=mm_r[:, kt, :],
                                             start=(kt == 0),
                                             stop=(kt == FKT - 1))
                    for m in range(4):
                        de = ppool.tile([P, BLK], ARD, tag="oe")
                        nc.vector.tensor_copy(de[:], ps_d[m][:])
                        nc.sync.dma_start(cc_d_in[mch * 4 + m], de[:])
                nc.gpsimd.collective_compute(
                    "AllReduce", ALU.add, replica_groups=rg,
                    ins=[cc_d_in.opt()], outs=[cc_d_out.opt()])
                pend_d[blk] = cc_d_out

            for l in range(L):
                co0 = phase_qkv_attn(l, 0)
                co1 = phase_qkv_attn(l, 1)
                phase_mlp(l, 0, co0)
                phase_mlp(l, 1, co1)
            apply_pending(0)
            apply_pending(1)

            # ================ final norm ================
            for blk in range(NBLK):
                tb = ds(blk * BLK, BLK)
                compute_rs(blk, 6, 7)
                for kt in range(KT):
                    xo = small.tile([P, BLK], F32, tag="tC")
                    nc.vector.scalar_tensor_tensor(xo[:], xT[:, kt, tb],
                                                   normw_sb[:, kt:kt + 1], rs_t[:],
                                                   op0=ALU.mult, op1=ALU.mult)
                    nc.sync.dma_start(XOUT[kt][:, tb], xo[:])

    nc.compile()
    return nc


def _rotary_tables():
    pos = np.arange(0, HD, 2, dtype=np.float32)
    inv_freq = 1.0 / (10000.0 ** (pos / HD))
    freqs = np.outer(np.arange(MAXPOS, dtype=np.float32), inv_freq)
    emb = np.concatenate([freqs, freqs], axis=-1)
    return np.cos(emb), np.sin(emb)


def kernel(tokens, position_ids, attention_mask, use_cache, k_cache, v_cache,
           embed, ln1, ln2, Wq, Wk, Wv, Wo, Wg, Wu, Wd, norm_w):
    import ml_dtypes
    from concourse.bass_utils import run_bass_kernel_spmd

    if 'nc' not in _BUILT:
        _BUILT['nc'] = _build()
    nc = _BUILT['nc']

    tokens = np.asarray(tokens)
    position_ids = np.asarray(position_ids)
    attention_mask = np.asarray(attention_mask)
    embed = np.asarray(embed, dtype=np.float32)
    ln1 = np.asarray(ln1, dtype=np.float32)
    ln2 = np.asarray(ln2, dtype=np.float32)
    Wq = np.asarray(Wq, dtype=np.float32); Wk = np.asarray(Wk, dtype=np.float32)
    Wv = np.asarray(Wv, dtype=np.float32); Wo = np.asarray(Wo, dtype=np.float32)
    Wg = np.asarray(Wg, dtype=np.float32); Wu = np.asarray(Wu, dtype=np.float32)
    Wd = np.asarray(Wd, dtype=np.float32)
    norm_w = np.asarray(norm_w, dtype=np.float32)

    cos_t, sin_t = _rotary_tables()

    # head-interleave permutation: tile m holds (head m, head m+4)
    perm = []
    for m in range(4):
        perm += list(range(HD * m, HD * m + HD))
        perm += list(range(HD * (m + 4), HD * (m + 4) + HD))
    perm = np.array(perm)

    def hvy(x):
        return x.astype(ml_dtypes.bfloat16) if HEAVY_BF16 else x

    def bf(x):
        return x.astype(ml_dtypes.bfloat16)

    in_maps = []
    for c in range(NCORES):
        b, t = c // TP, c % TP
        x0 = embed[tokens[b]]                      # [S, H]
        x0T = np.ascontiguousarray(x0.T.reshape(KT, P, S))
        cosb = cos_t[position_ids[b]]              # [S, 64]
        sinb = sin_t[position_ids[b]]
        cos128 = np.ascontiguousarray(np.tile(cosb.T, (2, 1)))  # [128, S]
        sin128 = np.ascontiguousarray(np.tile(sinb.T, (2, 1)))
        padb = ((1.0 - attention_mask[b].astype(np.float32)) * NEG)
        padb = np.ascontiguousarray(padb.reshape(SKT, P).T)     # [P, SKT]
        normw = np.ascontiguousarray(norm_w.reshape(KT, P).T)   # [P, KT]

        wq = (ln1[:, :, None] * Wq) / 8.0
        wq = wq[:, :, QD * t:QD * (t + 1)][:, :, perm]
        wq = np.ascontiguousarray(wq.reshape(L, KT, P, QD))
        wk = np.ascontiguousarray(
            (ln1[:, :, None] * Wk)[:, :, KVD * t:KVD * (t + 1)]
            .reshape(L, KT, P, KVD))
        wv = np.ascontiguousarray(
            (ln1[:, :, None] * Wv)[:, :, KVD * t:KVD * (t + 1)]
            .reshape(L, KT, P, KVD))
        wo = Wo[:, QD * t:QD * (t + 1), :][:, perm, :]
        wo = wo.reshape(L, QMT, P, 4, BLK).transpose(0, 3, 1, 2, 4)
        wo = np.ascontiguousarray(wo)
        wg = (ln2[:, :, None] * Wg)[:, :, FFS * t:FFS * (t + 1)]
        wg = wg.reshape(L, KT, P, 4, BLK).transpose(0, 3, 1, 2, 4)
        wg = np.ascontiguousarray(wg)
        wu = (ln2[:, :, None] * Wu)[:, :, FFS * t:FFS * (t + 1)]
        wu = wu.reshape(L, KT, P, 4, BLK).transpose(0, 3, 1, 2, 4)
        wu = np.ascontiguousarray(wu)
        wd = Wd[:, FFS * t:FFS * (t + 1), :]
        wd = wd.reshape(L, FKT, P, 4, BLK).transpose(0, 3, 1, 2, 4)
        wd = np.ascontiguousarray(wd)

        in_maps.append({
            "X0T": x0T, "COS": cos128, "SIN": sin128,
            "SINNEG": np.ascontiguousarray(-sin128), "PADB": padb,
            "NORMW": normw,
            "WQ": bf(wq), "WK": bf(wk), "WV": bf(wv),
            "WO": hvy(wo), "WG": hvy(wg), "WU": hvy(wu), "WD": hvy(wd),
        })

    res = run_bass_kernel_spmd(nc, in_maps, core_ids=list(range(NCORES)),
                               **_BUILT.get('run_kwargs', {}))
    _BUILT['last_res'] = res

    # ---- unshard ----
    xout = np.zeros((B, S, H), dtype=np.float32)
    k_out = np.zeros((L, B, NKV, S, HD), dtype=np.float32)
    v_out = np.zeros((L, B, NKV, S, HD), dtype=np.float32)
    for c in range(NCORES):
        b, t = c // TP, c % TP
        r = res.results[c]
        if t == 0:
            xo = r["XOUT"]  # [KT, P, S]
            xout[b] = xo.transpose(2, 0, 1).reshape(S, H)
        ko = r["KOUT"]  # [L, P, S]
        vo = r["VOUT"]
        for j in range(KVH):
            gh = KVH * t + j
            k_out[:, b, gh] = ko[:, HD * j:HD * (j + 1), :].transpose(0, 2, 1)
            v_out[:, b, gh] = vo[:, HD * j:HD * (j + 1), :].transpose(0, 2, 1)
    return xout, k_out, v_out
